# revision 1
# baseline (speedup 1.0000x reference)
"""Trainium2 Bass kernel for nn_MidAttnBlock (res-block -> full LxL attention -> res-block).

Contract: kernel(**inputs) takes the FULL inputs of reference.setup_inputs()
(x: (16,256,2048) f32, t: (16,256,1) f32, plus conv/groupnorm/linear params)
and returns the FULL (16,256,2048) f32 output.  Data-parallel over batch on
8 NeuronCores, 2 samples per core; each core runs an identical Bass program.

All heavy matmuls run in float32r (full-rate PE, ~bf16x2 precision).  The BIR
verifier requires f32r matmul operands to be *produced* as f32r, so every
tile feeding a matmul is allocated f32r and its producer writes it directly;
vector/scalar-engine readers go through a float32 bitcast.

Self-contained: all shapes/sharding hardcoded.
"""

import json as _json

import numpy as np

import concourse.bass as bass
import concourse.bass2jax as _b2j
import concourse.bass_utils as _bu
import concourse.tile as tile
from concourse import mybir
from concourse.vector_clock import ScopedClock, VectorClock


def _split_bir_waits(bir_json):
    """The walrus_driver in this container encodes at most ONE sync-wait per
    instruction (and none on Drain).  Tile's sem assigner attaches several.
    Rewrite the BIR: excess waits move to single-wait NoOps inserted directly
    before the instruction on the same engine."""
    m = _json.loads(bir_json)
    ctr = 0
    for fn in m.get("functions", []):
        for bb in fn.get("blocks", []):
            out = []
            for ins in bb.get("instructions", []):
                si = ins.get("sync_info")
                waits = (si or {}).get("on_wait") or []
                keep = 0 if ins.get("opcode") == "Drain" else 1
                if len(waits) > keep:
                    nmove = len(waits) - keep
                    for w in waits[:nmove]:
                        ctr += 1
                        out.append({
                            "debug": ins.get("debug", 0),
                            "engine": ins["engine"],
                            "ins": [],
                            "name": f"{ins['name']}-wsp{ctr}",
                            "opcode": "NoOp",
                            "outs": [],
                            "sync_info": {"on_update": [], "on_wait": [w]},
                        })
                    si["on_wait"] = waits[nmove:]
                out.append(ins)
            bb["instructions"] = out
    return _json.dumps(m).encode()


_orig_compile_bir_kernel = _bu.compile_bir_kernel


def _compile_bir_splitwaits(bir_json, tmpdir, neff_name="file.neff"):
    return _orig_compile_bir_kernel(_split_bir_waits(bir_json), tmpdir, neff_name)


if getattr(_bu.compile_bir_kernel, "__name__", "") != "_compile_bir_splitwaits":
    _bu.compile_bir_kernel = _compile_bir_splitwaits
    _b2j.compile_bir_kernel = _compile_bir_splitwaits


F32 = mybir.dt.float32
F32R = mybir.dt.float32r
AF = mybir.ActivationFunctionType
OP = mybir.AluOpType

P = 128          # partitions
C = 256          # channels
CB = 2           # channel blocks of 128
L = 2048         # sequence length
LS = 512         # l-slice (matmul moving dim)
NL = L // LS     # 4 slices
KB = L // P      # 16 k-blocks for attention
GPB = 16         # groups per channel-block (32 groups, 8 ch each)
EPS = 1e-5
S = 2            # samples per core
NCORES = 8
SCALE = 1.0 / 16.0  # 1/sqrt(C)


class _TileContextPatched(tile.TileContext):
    """TileContext whose kernel-tail drain carries no sem waits (the container
    walrus rejects waits on Drain); one SP NOP per proc carries them instead."""

    def _drain_and_barrier(self, tick_clock, wait_clock):
        gc = tick_clock.global_clock
        n = len(gc)
        for p in range(n):
            v = gc[p]
            if v > 0:
                vec = [0] * n
                vec[p] = v
                nop = self.nc.sync.nop()
                wait_clock.add_sem_waits(nop.ins, ScopedClock({None: VectorClock(vec)}))
        self.nc.sync.drain()
        self.nc.all_engine_barrier()
        assert self.sems is not None
        popped = self.nc._tile_sem_poison_stack.pop()
        assert popped is self._sem_poison
        self.nc.clear_and_free_semaphores(list(self.sems.allocated().values()))
        self.nc.all_engine_barrier()


def _f(ap):
    """Read an f32r tile as plain f32 (same bits) for VectorE/ScalarE inputs."""
    return ap.bitcast(F32)


def build_program(samples=S, use_bias=()):
    """Build the per-core Bass program (identical on all cores).

    use_bias: subset of {"c2b_r1", "c2b_r2", "linb"} enabling extra adds for
    biases that setup_inputs() keeps at zero.
    """
    nc = bass.Bass()

    # ---- DRAM I/O (per core) ----
    x_d = nc.dram_tensor("x", (samples, C, L), F32R, kind="ExternalInput")
    # t + conv1 bias, host-packed [samples, P, CB, 2(resblock)]
    t_d = nc.dram_tensor("tv", (samples, P, CB, 2), F32, kind="ExternalInput")
    w_conv = {}
    for rb in ("r1", "r2"):
        # host-packed [P(ic within block), icb, tap, oc]
        w_conv[rb, 1] = nc.dram_tensor(f"{rb}_w1t", (P, CB, 3, C), F32R, kind="ExternalInput")
        w_conv[rb, 2] = nc.dram_tensor(f"{rb}_w2t", (P, CB, 3, C), F32R, kind="ExternalInput")
    wkqv_d = nc.dram_tensor("wkqvt", (P, CB, 3 * C), F32R, kind="ExternalInput")
    gnw_d = {}
    for rb in ("r1", "r2"):
        for ln in (1, 2):
            gnw_d[rb, ln, "w"] = nc.dram_tensor(f"{rb}_gn{ln}_ws", (P, CB), F32, kind="ExternalInput")
            gnw_d[rb, ln, "b"] = nc.dram_tensor(f"{rb}_gn{ln}_bs", (P, CB), F32, kind="ExternalInput")
    c2b_d = {}
    if "c2b_r1" in use_bias:
        c2b_d["r1"] = nc.dram_tensor("r1_c2bs", (P, CB), F32, kind="ExternalInput")
    if "c2b_r2" in use_bias:
        c2b_d["r2"] = nc.dram_tensor("r2_c2bs", (P, CB), F32, kind="ExternalInput")
    linb_d = None
    if "linb" in use_bias:
        linb_d = nc.dram_tensor("lin_bs", (P, 3 * CB), F32, kind="ExternalInput")
    gind_d = nc.dram_tensor("gind", (P, GPB), F32R, kind="ExternalInput")  # 1/8 group indicator
    bind_d = nc.dram_tensor("bind", (CB, P, P), F32R, kind="ExternalInput")    # group->channel broadcast
    ones_d = nc.dram_tensor("ones", (P, 1), F32R, kind="ExternalInput")
    onesr_d = nc.dram_tensor("onesr", (1, P), F32R, kind="ExternalInput")
    out_d = nc.dram_tensor("out", (samples, C, L), F32, kind="ExternalOutput")

    with _TileContextPatched(nc) as tc, \
         tc.tile_pool(name="consts", bufs=1) as consts, \
         tc.tile_pool(name="padp", bufs=3) as padp, \
         tc.tile_pool(name="actp", bufs=2) as actp, \
         tc.tile_pool(name="seqp", bufs=1) as seqp, \
         tc.tile_pool(name="vtp", bufs=1) as vtp, \
         tc.tile_pool(name="expp", bufs=1) as expp, \
         tc.tile_pool(name="outp", bufs=2) as outp, \
         tc.tile_pool(name="rdbp", bufs=2) as rdbp, \
         tc.tile_pool(name="rdsp", bufs=1) as rdsp, \
         tc.tile_pool(name="small", bufs=4) as small, \
         tc.tile_pool(name="t2p", bufs=2) as t2p, \
         tc.tile_pool(name="pacc", bufs=4, space="PSUM") as pacc, \
         tc.tile_pool(name="pvec", bufs=2, space="PSUM") as pvec, \
         tc.tile_pool(name="prdb", bufs=2, space="PSUM") as prdb:

        # ---- persistent constants / weights in SBUF ----
        w1_sb = {}
        w2_sb = {}
        for rb in ("r1", "r2"):
            w1_sb[rb] = consts.tile([P, CB, 3, C], F32R, tag=f"w1_{rb}", name=f"w1_{rb}")
            nc.scalar.dma_start(w1_sb[rb][:], w_conv[rb, 1][:])
            w2_sb[rb] = consts.tile([P, CB, 3, C], F32R, tag=f"w2_{rb}", name=f"w2_{rb}")
            nc.gpsimd.dma_start(w2_sb[rb][:], w_conv[rb, 2][:])
        wkqv_sb = consts.tile([P, CB, 3 * C], F32R, tag="wkqv", name="wkqv")
        nc.scalar.dma_start(wkqv_sb[:], wkqv_d[:])
        gnp_sb = {}
        for rb in ("r1", "r2"):
            for ln in (1, 2):
                for wb in ("w", "b"):
                    tl = consts.tile([P, CB], F32, tag=f"gn_{rb}{ln}{wb}", name=f"gn_{rb}{ln}{wb}")
                    nc.gpsimd.dma_start(tl[:], gnw_d[rb, ln, wb][:])
                    gnp_sb[rb, ln, wb] = tl
        c2b_sb = {}
        for rb, d in c2b_d.items():
            c2b_sb[rb] = consts.tile([P, CB], F32, tag=f"c2b_{rb}", name=f"c2b_{rb}")
            nc.gpsimd.dma_start(c2b_sb[rb][:], d[:])
        linb_sb = None
        if linb_d is not None:
            linb_sb = consts.tile([P, 3 * CB], F32, tag="linb", name="linb")
            nc.gpsimd.dma_start(linb_sb[:], linb_d[:])
        gind_sb = consts.tile([P, GPB], F32R, tag="gind", name="gind")
        nc.gpsimd.dma_start(gind_sb[:], gind_d[:])
        bind_sb = consts.tile([P, CB, P], F32R, tag="bind", name="bind")
        nc.gpsimd.dma_start(bind_sb[:], bind_d.rearrange("cb p c -> p cb c"))
        ones_sb = consts.tile([P, 1], F32R, tag="ones", name="ones")
        nc.gpsimd.dma_start(ones_sb[:], ones_d[:])
        onesr_sb = consts.tile([1, P], F32R, tag="onesr", name="onesr")
        nc.gpsimd.dma_start(onesr_sb[:], onesr_d[:])
        eps_sb = consts.tile([P, 1], F32, tag="eps", name="eps")
        nc.vector.memset(eps_sb[:], EPS)
        zero2 = consts.tile([P, 2], F32, tag="zero2", name="zero2")
        nc.vector.memset(zero2[:], 0.0)

        def alloc_padded(tag, pool):
            """[P, L+2] f32r tile per channel block; data cols [1, L+1), zero edges."""
            ts = []
            for cb in range(CB):
                tl = pool.tile([P, L + 2], F32R, tag=f"{tag}{cb}", name=f"{tag}{cb}")
                nc.vector.tensor_copy(out=tl[:, 0:1], in_=zero2[:, 0:1])
                nc.vector.tensor_copy(out=tl[:, L + 1 : L + 2], in_=zero2[:, 0:1])
                ts.append(tl)
            return ts

        def gn_relu(src, dst, rb, ln):
            """dst = relu(groupnorm(src) * w + b); src/dst are padded f32r pairs.

            One merged nonlinear chain over all 32 groups, then per-block
            broadcast and NL-chunked scalar-engine applies."""
            gp = []
            for cb in range(CB):
                stats = small.tile([P, NL, 6], F32, tag="stats", name="stats")
                for i in range(NL):
                    nc.vector.bn_stats(out=stats[:, i, :], in_=_f(src[cb][:, 1 + i * LS : 1 + (i + 1) * LS]))
                mv = small.tile([P, 2], F32, tag="mv", name="mv")
                nc.vector.bn_aggr(out=mv[:], in_=stats[:])
                # tmp = [mean_c, E[x^2]_c]  (f32r: feeds the aggregation matmul)
                tmp = small.tile([P, 2], F32R, tag="tmp", name="tmp")
                nc.vector.tensor_copy(out=tmp[:, 0:1], in_=mv[:, 0:1])
                nc.vector.tensor_tensor(out=tmp[:, 1:2], in0=mv[:, 0:1], in1=mv[:, 0:1], op=OP.mult)
                nc.vector.tensor_tensor(out=tmp[:, 1:2], in0=_f(tmp[:, 1:2]), in1=mv[:, 1:2], op=OP.add)
                g = pvec.tile([GPB, 2], F32, tag="vec", name="gp")
                nc.tensor.matmul(g[:], gind_sb[:], tmp[:], start=True, stop=True)
                gp.append(g)
            # merged group stats; block-cb groups live at partition offset 32*cb
            NG = 32 * CB
            gs = small.tile([NG, 2], F32, tag="gs", name="gs")
            nc.vector.tensor_copy(out=gs[:], in_=zero2[:NG])
            for cb in range(CB):
                nc.vector.tensor_copy(out=gs[cb * 32 : cb * 32 + GPB, :], in_=gp[cb][:])
            var = small.tile([NG, 1], F32, tag="var", name="var")
            nc.vector.tensor_tensor(out=var[:], in0=gs[:, 0:1], in1=gs[:, 0:1], op=OP.mult)
            nc.vector.tensor_tensor(out=var[:], in0=gs[:, 1:2], in1=var[:], op=OP.subtract)
            nc.scalar.activation(out=var[:], in_=var[:], func=AF.Ln, bias=eps_sb[:NG])
            rstd = small.tile([NG, 1], F32, tag="rstd", name="rstd")
            nc.scalar.activation(out=rstd[:], in_=var[:], func=AF.Exp, scale=-0.5)
            # pack [rstd_g, -m_g], zero-extended to 128 partitions
            gpk = small.tile([P, 2], F32R, tag="gpk", name="gpk")
            nc.vector.tensor_copy(out=gpk[:], in_=zero2[:])
            nc.vector.tensor_copy(out=gpk[:NG, 0:1], in_=rstd[:])
            nc.vector.tensor_scalar_mul(gpk[:NG, 1:2], gs[:, 0:1], -1.0)
            for cb in range(CB):
                # broadcast to channels: bc[c, :] = [rstd_g(c), -m_g(c)]
                bc = pvec.tile([P, 2], F32, tag="vec", name="bc")
                nc.tensor.matmul(bc[:], bind_sb[:, cb, :], gpk[:], start=True, stop=True)
                sb = small.tile([P, 2], F32, tag="sb", name="sb")
                # s = rstd*w ; b2 = b - m*s
                nc.vector.tensor_scalar_mul(sb[:, 0:1], bc[:, 0:1], gnp_sb[rb, ln, "w"][:, cb : cb + 1])
                nc.vector.tensor_tensor(out=sb[:, 1:2], in0=bc[:, 1:2], in1=sb[:, 0:1], op=OP.mult)
                nc.vector.tensor_scalar_add(sb[:, 1:2], sb[:, 1:2], gnp_sb[rb, ln, "b"][:, cb : cb + 1])
                # apply + relu on ScalarE in NL chunks so convs can start early
                for i in range(NL):
                    nc.scalar.activation(
                        out=dst[cb][:, 1 + i * LS : 1 + (i + 1) * LS],
                        in_=_f(src[cb][:, 1 + i * LS : 1 + (i + 1) * LS]),
                        func=AF.Relu,
                        bias=sb[:, 1:2],
                        scale=sb[:, 0:1],
                    )

        def conv3(src, wt, consume):
            """3-tap conv over padded f32r src; consume(ocb, ls, psum_tile)."""
            for ocb in range(CB):
                for ls in range(NL):
                    ps = pacc.tile([P, LS], F32, tag="acc", name="acc")
                    k = 0
                    for icb in range(CB):
                        for tap in range(3):
                            nc.tensor.matmul(
                                ps[:],
                                wt[:, icb, tap, ocb * P : (ocb + 1) * P],
                                src[icb][:, ls * LS + tap : ls * LS + tap + LS],
                                start=(k == 0),
                                stop=(k == 5),
                            )
                            k += 1
                    consume(ocb, ls, ps)

        for s in range(samples):
            with nc.named_scope(f"s{s}_load"):
                xp = alloc_padded("pad", padp)
                for cb in range(CB):
                    for i in range(NL):
                        nc.sync.dma_start(
                            xp[cb][:, 1 + i * LS : 1 + (i + 1) * LS],
                            x_d[s, cb * P : (cb + 1) * P, i * LS : (i + 1) * LS],
                        )
                t2 = t2p.tile([P, CB, 2], F32, tag="t2", name="t2")
                nc.sync.dma_start(t2[:], t_d[s])

            def res_block(rb, rbi, src, final):
                """src: padded f32r pair.  final=False: return x+conv2(...) f32r pair;
                final=True: stream x+conv2(...) to DRAM out."""
                a = alloc_padded("act", actp)
                with nc.named_scope(f"s{s}_{rb}_gn1"):
                    gn_relu(src, a, rb, 1)
                h = alloc_padded("pad", padp)
                with nc.named_scope(f"s{s}_{rb}_conv1"):
                    def eat1(ocb, ls, ps):
                        nc.any.tensor_scalar_add(
                            h[ocb][:, 1 + ls * LS : 1 + (ls + 1) * LS], ps[:],
                            t2[:, ocb, rbi : rbi + 1],
                        )
                    conv3(a, w1_sb[rb], eat1)
                a2 = alloc_padded("act", actp)
                with nc.named_scope(f"s{s}_{rb}_gn2"):
                    gn_relu(h, a2, rb, 2)
                res = None
                if not final:
                    res = [seqp.tile([P, L], F32R, tag=f"res{cb}", name=f"res{cb}") for cb in range(CB)]
                with nc.named_scope(f"s{s}_{rb}_conv2"):
                    def eat2(ocb, ls, ps):
                        if rb in c2b_sb:
                            nc.vector.tensor_scalar_add(ps[:], ps[:], c2b_sb[rb][:, ocb : ocb + 1])
                        resid = _f(src[ocb][:, 1 + ls * LS : 1 + (ls + 1) * LS])
                        if final:
                            ot = outp.tile([P, LS], F32, tag="out", name="ot")
                            nc.vector.tensor_tensor(out=ot[:], in0=ps[:], in1=resid, op=OP.add)
                            nc.sync.dma_start(
                                out_d[s, ocb * P : (ocb + 1) * P, ls * LS : (ls + 1) * LS], ot[:]
                            )
                        else:
                            nc.any.tensor_tensor(
                                out=res[ocb][:, ls * LS : (ls + 1) * LS],
                                in0=ps[:], in1=resid, op=OP.add,
                            )
                    conv3(a2, w2_sb[rb], eat2)
                return res

            x1 = res_block("r1", 0, xp, final=False)

            # ---- kqv projections (lin_w rows: [k | q | v]) ----
            kt = [seqp.tile([P, L], F32R, tag=f"kt{cb}", name=f"kt{cb}") for cb in range(CB)]
            qt = [seqp.tile([P, L], F32R, tag=f"qt{cb}", name=f"qt{cb}") for cb in range(CB)]
            vt = vtp.tile([P, KB, C], F32R, tag="vt", name="vt")
            with nc.named_scope(f"s{s}_kqv"):
                for j, dst in ((0, kt), (1, qt)):
                    for ocb in range(CB):
                        off = j * C + ocb * P
                        for ls in range(NL):
                            ps = pacc.tile([P, LS], F32, tag="acc", name="acc")
                            for icb in range(CB):
                                nc.tensor.matmul(
                                    ps[:],
                                    wkqv_sb[:, icb, off : off + P],
                                    x1[icb][:, ls * LS : (ls + 1) * LS],
                                    start=(icb == 0),
                                    stop=(icb == 1),
                                )
                            dsl = dst[ocb][:, ls * LS : (ls + 1) * LS]
                            if linb_sb is not None:
                                nc.vector.tensor_scalar_add(
                                    dsl, ps[:], linb_sb[:, j * CB + ocb : j * CB + ocb + 1]
                                )
                            else:
                                nc.any.tensor_copy(out=dsl, in_=ps[:])
                # vT[l, c] (l on partitions) for the attention output matmul
                for lb in range(KB):
                    ps = pacc.tile([P, LS], F32, tag="acc", name="acc")
                    for icb in range(CB):
                        nc.tensor.matmul(
                            ps[:, :C],
                            x1[icb][:, lb * P : (lb + 1) * P],
                            wkqv_sb[:, icb, 2 * C : 3 * C],
                            start=(icb == 0),
                            stop=(icb == 1),
                        )
                    nc.any.tensor_copy(out=vt[:, lb, :], in_=ps[:, :C])

            # ---- attention: softmax over k (no max-subtract; scores are O(5)) ----
            av = alloc_padded("pad", padp)
            for qs in range(NL):
                with nc.named_scope(f"s{s}_attn{qs}"):
                    dn = pvec.tile([1, LS], F32, tag="vec", name="dn")
                    psav = [pacc.tile([P, LS], F32, tag="acc", name="psav") for _ in range(CB)]
                    KBH = KB // 2
                    for kh in range(2):
                        ex = expp.tile([P, KBH, LS], F32R, tag="exp", name="exp")
                        for kb in range(KBH):
                            kbg = kh * KBH + kb
                            ps = pacc.tile([P, LS], F32, tag="acc", name="acc")
                            for cb in range(CB):
                                nc.tensor.matmul(
                                    ps[:],
                                    kt[cb][:, kbg * P : (kbg + 1) * P],
                                    qt[cb][:, qs * LS : (qs + 1) * LS],
                                    start=(cb == 0),
                                    stop=(cb == 1),
                                )
                            nc.scalar.activation(out=ex[:, kb, :], in_=ps[:], func=AF.Exp, scale=SCALE)
                            nc.tensor.matmul(
                                dn[:], ones_sb[:], ex[:, kb, :],
                                start=(kbg == 0), stop=(kbg == KB - 1),
                            )
                        for cb in range(CB):
                            for kb in range(KBH):
                                kbg = kh * KBH + kb
                                nc.tensor.matmul(
                                    psav[cb][:],
                                    vt[:, kbg, cb * P : (cb + 1) * P],
                                    ex[:, kb, :],
                                    start=(kbg == 0),
                                    stop=(kbg == KB - 1),
                                )
                    lnd = rdsp.tile([1, LS], F32, tag="lnd", name="lnd")
                    nc.scalar.activation(out=lnd[:], in_=dn[:], func=AF.Ln)
                    rd = rdsp.tile([1, LS], F32R, tag="rd", name="rd")
                    nc.scalar.activation(out=rd[:], in_=lnd[:], func=AF.Exp, scale=-1.0)
                    # broadcast 1/denom across partitions via K=1 ones-matmul
                    rb_ps = prdb.tile([P, LS], F32, tag="rdb", name="rb_ps")
                    nc.tensor.matmul(rb_ps[:], onesr_sb[:], rd[:], start=True, stop=True)
                    rdb = rdbp.tile([P, LS], F32, tag="rdbs", name="rdb")
                    nc.scalar.activation(out=rdb[:], in_=rb_ps[:], func=AF.Copy)
                    for cb in range(CB):
                        avs = av[cb][:, 1 + qs * LS : 1 + (qs + 1) * LS]
                        nc.vector.tensor_tensor(out=avs, in0=psav[cb][:], in1=rdb[:], op=OP.mult)
                        if linb_sb is not None:
                            nc.vector.tensor_scalar_add(
                                avs, _f(avs), linb_sb[:, 2 * CB + cb : 2 * CB + cb + 1]
                            )

            res_block("r2", 1, av, final=True)

    nc.finalize()
    return nc


def _pack_conv_w(w):
    """(O, I, 3) -> [P, icb, tap, oc]."""
    w = np.asarray(w, dtype=np.float32)
    o, i, k = w.shape
    return np.ascontiguousarray(w.transpose(1, 2, 0).reshape(CB, P, 3, o).transpose(1, 0, 2, 3))


def _pack_gn(v):
    """(256,) -> [P, CB]"""
    return np.ascontiguousarray(np.asarray(v, dtype=np.float32).reshape(CB, P).T)


def make_in_maps(inp, use_bias):
    """Host-side packing; returns the per-core input maps."""
    gind = np.zeros((P, GPB), np.float32)
    bind = np.zeros((CB, P, P), np.float32)
    for cc in range(P):
        gind[cc, cc // 8] = 0.125
        for cb in range(CB):
            bind[cb, cb * 32 + cc // 8, cc] = 1.0
    shared = {
        "wkqvt": np.ascontiguousarray(
            inp["lin_w"][:, :, 0].T.reshape(CB, P, 3 * C).transpose(1, 0, 2)
        ),
        "gind": gind,
        "bind": bind,
        "ones": np.ones((P, 1), np.float32),
        "onesr": np.ones((1, P), np.float32),
    }
    for rb in ("r1", "r2"):
        shared[f"{rb}_w1t"] = _pack_conv_w(inp[f"{rb}_c1_w"])
        shared[f"{rb}_w2t"] = _pack_conv_w(inp[f"{rb}_c2_w"])
        for ln in (1, 2):
            shared[f"{rb}_gn{ln}_ws"] = _pack_gn(inp[f"{rb}_gn{ln}_w"])
            shared[f"{rb}_gn{ln}_bs"] = _pack_gn(inp[f"{rb}_gn{ln}_b"])
    if "c2b_r1" in use_bias:
        shared["r1_c2bs"] = _pack_gn(inp["r1_c2_b"])
    if "c2b_r2" in use_bias:
        shared["r2_c2bs"] = _pack_gn(inp["r2_c2_b"])
    if "linb" in use_bias:
        shared["lin_bs"] = np.ascontiguousarray(inp["lin_b"].reshape(3 * CB, P).T)

    # per-sample conv1 bias vector: t[s] + c1_b per res block -> [P, CB, 2]
    tfull = inp["t"][:, :, 0]  # (B, C)
    nb = inp["x"].shape[0]
    tv = np.empty((nb, P, CB, 2), np.float32)
    for rbi, rb in enumerate(("r1", "r2")):
        v = tfull + inp[f"{rb}_c1_b"][None, :]
        tv[:, :, :, rbi] = v.reshape(nb, CB, P).transpose(0, 2, 1)

    in_maps = []
    for c in range(NCORES):
        sl = slice(S * c, S * (c + 1))
        m = dict(shared)
        m["x"] = inp["x"][sl]
        m["tv"] = np.ascontiguousarray(tv[sl])
        in_maps.append(m)
    return in_maps


_CACHE = {}


def kernel(**inputs):
    inp = {k: np.ascontiguousarray(np.asarray(v, dtype=np.float32)) for k, v in inputs.items()}

    use_bias = []
    if np.any(inp["r1_c2_b"]):
        use_bias.append("c2b_r1")
    if np.any(inp["r2_c2_b"]):
        use_bias.append("c2b_r2")
    if np.any(inp["lin_b"]):
        use_bias.append("linb")
    use_bias = tuple(use_bias)

    if ("nc", use_bias) not in _CACHE:
        _CACHE[("nc", use_bias)] = build_program(S, use_bias)
    nc = _CACHE[("nc", use_bias)]

    in_maps = make_in_maps(inp, use_bias)
    res = _bu.run_bass_kernel_spmd(nc, in_maps, core_ids=list(range(NCORES)))
    out = np.concatenate([res.results[c]["out"] for c in range(NCORES)], axis=0)
    return out.astype(np.float32)



# revision 6
# speedup vs baseline: 1.2530x; 1.2530x over previous
"""Trainium2 Bass kernel for nn_MidAttnBlock (res-block -> full LxL attention -> res-block).

Contract: kernel(**inputs) takes the FULL inputs of reference.setup_inputs()
(x: (16,256,2048) f32, t: (16,256,1) f32, plus conv/groupnorm/linear params)
and returns the FULL (16,256,2048) f32 output.  Data-parallel over batch on
8 NeuronCores, 2 samples per core; each core runs an identical Bass program.

All heavy matmuls run in bfloat16: the PE streams the moving operand at
2 bytes/cycle/partition, so bf16 runs at 1 cycle/row vs float32r's 2.
Residual-carrying tiles (x, h, av) stay f32; accumulation is f32 in PSUM.

The attention computes av^T = ex^T @ [v | 1] with the exp tiles as the
stationary operand: the softmax denominator falls out as column 256 of the
av^T accumulator (no separate ones-matmul), the normalize is a per-partition
scale fused into the PSUM->SBUF copy, and the [q,c] -> [c,q] transpose back
is a cheap N=128 identity matmul per 128x128 block.

Self-contained: all shapes/sharding hardcoded.
"""

import json as _json

import ml_dtypes as _mld
import numpy as np

import concourse.bass as bass
import concourse.bass2jax as _b2j
import concourse.bass_utils as _bu
import concourse.tile as tile
from concourse import mybir
from concourse.vector_clock import ScopedClock, VectorClock


def _split_bir_waits(bir_json):
    """The walrus_driver in this container encodes at most ONE sync-wait per
    instruction (and none on Drain).  Tile's sem assigner attaches several.
    Rewrite the BIR: excess waits move to single-wait NoOps inserted directly
    before the instruction on the same engine."""
    m = _json.loads(bir_json)
    ctr = 0
    for fn in m.get("functions", []):
        for bb in fn.get("blocks", []):
            out = []
            for ins in bb.get("instructions", []):
                si = ins.get("sync_info")
                waits = (si or {}).get("on_wait") or []
                keep = 0 if ins.get("opcode") == "Drain" else 1
                if len(waits) > keep:
                    nmove = len(waits) - keep
                    for w in waits[:nmove]:
                        ctr += 1
                        out.append({
                            "debug": ins.get("debug", 0),
                            "engine": ins["engine"],
                            "ins": [],
                            "name": f"{ins['name']}-wsp{ctr}",
                            "opcode": "NoOp",
                            "outs": [],
                            "sync_info": {"on_update": [], "on_wait": [w]},
                        })
                    si["on_wait"] = waits[nmove:]
                out.append(ins)
            bb["instructions"] = out
    return _json.dumps(m).encode()


_orig_compile_bir_kernel = _bu.compile_bir_kernel


def _compile_bir_splitwaits(bir_json, tmpdir, neff_name="file.neff"):
    return _orig_compile_bir_kernel(_split_bir_waits(bir_json), tmpdir, neff_name)


if getattr(_bu.compile_bir_kernel, "__name__", "") != "_compile_bir_splitwaits":
    _bu.compile_bir_kernel = _compile_bir_splitwaits
    _b2j.compile_bir_kernel = _compile_bir_splitwaits


F32 = mybir.dt.float32
F32R = mybir.dt.float32r
BF16 = mybir.dt.bfloat16
AF = mybir.ActivationFunctionType
OP = mybir.AluOpType

P = 128          # partitions
C = 256          # channels
CB = 2           # channel blocks of 128
L = 2048         # sequence length
LS = 512         # l-slice (matmul moving dim)
NL = L // LS     # 4 slices
AC = 1024        # gn-apply chunk
NA = L // AC     # 2 apply chunks
KB = L // P      # 16 k-blocks for attention
QB = LS // P     # 4 q-blocks per l-slice
GPB = 16         # groups per channel-block (32 groups, 8 ch each)
EPS = 1e-5
S = 2            # samples per core
NCORES = 8
SCALE = 1.0 / 16.0  # 1/sqrt(C)


class _TileContextPatched(tile.TileContext):
    """TileContext whose kernel-tail drain carries no sem waits (the container
    walrus rejects waits on Drain); one SP NOP per proc carries them instead."""

    def _drain_and_barrier(self, tick_clock, wait_clock):
        gc = tick_clock.global_clock
        n = len(gc)
        for p in range(n):
            v = gc[p]
            if v > 0:
                vec = [0] * n
                vec[p] = v
                nop = self.nc.sync.nop()
                wait_clock.add_sem_waits(nop.ins, ScopedClock({None: VectorClock(vec)}))
        self.nc.sync.drain()
        self.nc.all_engine_barrier()
        assert self.sems is not None
        popped = self.nc._tile_sem_poison_stack.pop()
        assert popped is self._sem_poison
        self.nc.clear_and_free_semaphores(list(self.sems.allocated().values()))
        self.nc.all_engine_barrier()


def build_program(samples=S, use_bias=()):
    """Build the per-core Bass program (identical on all cores).

    use_bias: subset of {"c2b_r1", "c2b_r2", "linb"} enabling extra adds for
    biases that setup_inputs() keeps at zero.
    """
    nc = bass.Bass()

    # ---- DRAM I/O (per core) ----
    x_d = nc.dram_tensor("x", (samples, C, L), F32, kind="ExternalInput")
    # t + conv1 bias, host-packed [samples, P, CB, 2(resblock)]
    t_d = nc.dram_tensor("tv", (samples, P, CB, 2), F32, kind="ExternalInput")
    w_conv = {}
    for rb in ("r1", "r2"):
        # host-packed [P(ic within block), icb, tap, oc]
        w_conv[rb, 1] = nc.dram_tensor(f"{rb}_w1t", (P, CB, 3, C), BF16, kind="ExternalInput")
        w_conv[rb, 2] = nc.dram_tensor(f"{rb}_w2t", (P, CB, 3, C), BF16, kind="ExternalInput")
    wkqv_d = nc.dram_tensor("wkqvt", (P, CB, 3 * C), BF16, kind="ExternalInput")
    gnw_d = {}
    for rb in ("r1", "r2"):
        for ln in (1, 2):
            gnw_d[rb, ln, "w"] = nc.dram_tensor(f"{rb}_gn{ln}_ws", (P, CB), F32, kind="ExternalInput")
            gnw_d[rb, ln, "b"] = nc.dram_tensor(f"{rb}_gn{ln}_bs", (P, CB), F32, kind="ExternalInput")
    c2b_d = {}
    if "c2b_r1" in use_bias:
        c2b_d["r1"] = nc.dram_tensor("r1_c2bs", (P, CB), F32, kind="ExternalInput")
    if "c2b_r2" in use_bias:
        c2b_d["r2"] = nc.dram_tensor("r2_c2bs", (P, CB), F32, kind="ExternalInput")
    linb_d = None
    if "linb" in use_bias:
        linb_d = nc.dram_tensor("lin_bs", (P, 3 * CB), F32, kind="ExternalInput")
    gind_d = nc.dram_tensor("gind", (P, GPB), F32R, kind="ExternalInput")  # 1/8 group indicator
    bind_d = nc.dram_tensor("bind", (CB, P, P), F32R, kind="ExternalInput")    # group->channel broadcast
    ident_d = nc.dram_tensor("ident", (P, P), BF16, kind="ExternalInput")
    out_d = nc.dram_tensor("out", (samples, C, L), F32, kind="ExternalOutput")

    with _TileContextPatched(nc) as tc, \
         tc.tile_pool(name="consts", bufs=1) as consts, \
         tc.tile_pool(name="padp", bufs=3) as padp, \
         tc.tile_pool(name="actp", bufs=2) as actp, \
         tc.tile_pool(name="seqp", bufs=1) as seqp, \
         tc.tile_pool(name="vtp", bufs=1) as vtp, \
         tc.tile_pool(name="expp", bufs=2) as expp, \
         tc.tile_pool(name="outp", bufs=2) as outp, \
         tc.tile_pool(name="avnp", bufs=4) as avnp, \
         tc.tile_pool(name="small", bufs=4) as small, \
         tc.tile_pool(name="t2p", bufs=2) as t2p, \
         tc.tile_pool(name="pacc", bufs=4, space="PSUM") as pacc, \
         tc.tile_pool(name="pvec", bufs=2, space="PSUM") as pvec, \
         tc.tile_pool(name="ptr", bufs=1, space="PSUM") as ptr:

        # ---- persistent constants / weights in SBUF ----
        w1_sb = {}
        w2_sb = {}
        for rb in ("r1", "r2"):
            w1_sb[rb] = consts.tile([P, CB, 3, C], BF16, tag=f"w1_{rb}", name=f"w1_{rb}")
            nc.scalar.dma_start(w1_sb[rb][:], w_conv[rb, 1][:])
            w2_sb[rb] = consts.tile([P, CB, 3, C], BF16, tag=f"w2_{rb}", name=f"w2_{rb}")
            nc.gpsimd.dma_start(w2_sb[rb][:], w_conv[rb, 2][:])
        wkqv_sb = consts.tile([P, CB, 3 * C], BF16, tag="wkqv", name="wkqv")
        nc.scalar.dma_start(wkqv_sb[:], wkqv_d[:])
        gnp_sb = {}
        for rb in ("r1", "r2"):
            for ln in (1, 2):
                for wb in ("w", "b"):
                    tl = consts.tile([P, CB], F32, tag=f"gn_{rb}{ln}{wb}", name=f"gn_{rb}{ln}{wb}")
                    nc.gpsimd.dma_start(tl[:], gnw_d[rb, ln, wb][:])
                    gnp_sb[rb, ln, wb] = tl
        c2b_sb = {}
        for rb, d in c2b_d.items():
            c2b_sb[rb] = consts.tile([P, CB], F32, tag=f"c2b_{rb}", name=f"c2b_{rb}")
            nc.gpsimd.dma_start(c2b_sb[rb][:], d[:])
        linb_sb = None
        if linb_d is not None:
            linb_sb = consts.tile([P, 3 * CB], F32, tag="linb", name="linb")
            nc.gpsimd.dma_start(linb_sb[:], linb_d[:])
        gind_sb = consts.tile([P, GPB], F32R, tag="gind", name="gind")
        nc.gpsimd.dma_start(gind_sb[:], gind_d[:])
        bind_sb = consts.tile([P, CB, P], F32R, tag="bind", name="bind")
        nc.gpsimd.dma_start(bind_sb[:], bind_d.rearrange("cb p c -> p cb c"))
        ident_sb = consts.tile([P, P], BF16, tag="ident", name="ident")
        nc.scalar.dma_start(ident_sb[:], ident_d[:])
        eps_sb = consts.tile([P, 1], F32, tag="eps", name="eps")
        nc.vector.memset(eps_sb[:], EPS)
        zero2 = consts.tile([P, 2], F32, tag="zero2", name="zero2")
        nc.vector.memset(zero2[:], 0.0)
        zero2b = consts.tile([P, 2], BF16, tag="zero2b", name="zero2b")
        nc.vector.memset(zero2b[:], 0.0)

        def alloc_padded(tag, pool, dt, zt):
            """[P, L+2] tile per channel block; data cols [1, L+1), zero edges."""
            ts = []
            for cb in range(CB):
                tl = pool.tile([P, L + 2], dt, tag=f"{tag}{cb}", name=f"{tag}{cb}")
                nc.vector.tensor_copy(out=tl[:, 0:1], in_=zt[:, 0:1])
                nc.vector.tensor_copy(out=tl[:, L + 1 : L + 2], in_=zt[:, 0:1])
                ts.append(tl)
            return ts

        def gn_relu(src, dst, rb, ln):
            """dst = relu(groupnorm(src) * w + b); src padded f32, dst padded bf16.

            One merged nonlinear chain over all 32 groups, then per-block
            broadcast and NA-chunked scalar-engine applies."""
            gp = []
            for cb in range(CB):
                stats = small.tile([P, NL, 6], F32, tag="stats", name="stats")
                for i in range(NL):
                    nc.vector.bn_stats(out=stats[:, i, :], in_=src[cb][:, 1 + i * LS : 1 + (i + 1) * LS])
                mv = small.tile([P, 2], F32, tag="mv", name="mv")
                nc.vector.bn_aggr(out=mv[:], in_=stats[:])
                # tmp = [mean_c, E[x^2]_c]  (f32r: feeds the aggregation matmul)
                tmp = small.tile([P, 2], F32R, tag="tmp", name="tmp")
                nc.vector.tensor_copy(out=tmp[:, 0:1], in_=mv[:, 0:1])
                nc.vector.tensor_tensor(out=tmp[:, 1:2], in0=mv[:, 0:1], in1=mv[:, 0:1], op=OP.mult)
                nc.vector.tensor_tensor(out=tmp[:, 1:2], in0=tmp[:, 1:2].bitcast(F32), in1=mv[:, 1:2], op=OP.add)
                g = pvec.tile([GPB, 2], F32, tag="vec", name="gp")
                nc.tensor.matmul(g[:], gind_sb[:], tmp[:], start=True, stop=True)
                gp.append(g)
            # merged group stats; block-cb groups live at partition offset 32*cb
            NG = 32 * CB
            gs = small.tile([NG, 2], F32, tag="gs", name="gs")
            nc.vector.tensor_copy(out=gs[:], in_=zero2[:NG])
            for cb in range(CB):
                nc.vector.tensor_copy(out=gs[cb * 32 : cb * 32 + GPB, :], in_=gp[cb][:])
            var = small.tile([NG, 1], F32, tag="var", name="var")
            nc.vector.tensor_tensor(out=var[:], in0=gs[:, 0:1], in1=gs[:, 0:1], op=OP.mult)
            nc.vector.tensor_tensor(out=var[:], in0=gs[:, 1:2], in1=var[:], op=OP.subtract)
            nc.scalar.activation(out=var[:], in_=var[:], func=AF.Ln, bias=eps_sb[:NG])
            rstd = small.tile([NG, 1], F32, tag="rstd", name="rstd")
            nc.scalar.activation(out=rstd[:], in_=var[:], func=AF.Exp, scale=-0.5)
            # pack [rstd_g, -m_g], zero-extended to 128 partitions
            gpk = small.tile([P, 2], F32R, tag="gpk", name="gpk")
            nc.vector.tensor_copy(out=gpk[:], in_=zero2[:])
            nc.vector.tensor_copy(out=gpk[:NG, 0:1], in_=rstd[:])
            nc.vector.tensor_scalar_mul(gpk[:NG, 1:2], gs[:, 0:1], -1.0)
            for cb in range(CB):
                # broadcast to channels: bc[c, :] = [rstd_g(c), -m_g(c)]
                bc = pvec.tile([P, 2], F32, tag="vec", name="bc")
                nc.tensor.matmul(bc[:], bind_sb[:, cb, :], gpk[:], start=True, stop=True)
                sb = small.tile([P, 2], F32, tag="sb", name="sb")
                # s = rstd*w ; b2 = b - m*s
                nc.vector.tensor_scalar_mul(sb[:, 0:1], bc[:, 0:1], gnp_sb[rb, ln, "w"][:, cb : cb + 1])
                nc.vector.tensor_tensor(out=sb[:, 1:2], in0=bc[:, 1:2], in1=sb[:, 0:1], op=OP.mult)
                nc.vector.tensor_scalar_add(sb[:, 1:2], sb[:, 1:2], gnp_sb[rb, ln, "b"][:, cb : cb + 1])
                # apply + relu on ScalarE in NA chunks so convs can start early
                for i in range(NA):
                    nc.scalar.activation(
                        out=dst[cb][:, 1 + i * AC : 1 + (i + 1) * AC],
                        in_=src[cb][:, 1 + i * AC : 1 + (i + 1) * AC],
                        func=AF.Relu,
                        bias=sb[:, 1:2],
                        scale=sb[:, 0:1],
                    )

        def conv3(src, wt, consume):
            """3-tap conv over padded bf16 src; consume(ocb, ls, psum_tile)."""
            for ocb in range(CB):
                for ls in range(NL):
                    ps = pacc.tile([P, LS], F32, tag="acc", name="acc")
                    k = 0
                    for icb in range(CB):
                        for tap in range(3):
                            nc.tensor.matmul(
                                ps[:],
                                wt[:, icb, tap, ocb * P : (ocb + 1) * P],
                                src[icb][:, ls * LS + tap : ls * LS + tap + LS],
                                start=(k == 0),
                                stop=(k == 5),
                            )
                            k += 1
                    consume(ocb, ls, ps)

        for s in range(samples):
            with nc.named_scope(f"s{s}_load"):
                xp = alloc_padded("pad", padp, F32, zero2)
                for cb in range(CB):
                    for i in range(NL):
                        nc.sync.dma_start(
                            xp[cb][:, 1 + i * LS : 1 + (i + 1) * LS],
                            x_d[s, cb * P : (cb + 1) * P, i * LS : (i + 1) * LS],
                        )
                t2 = t2p.tile([P, CB, 2], F32, tag="t2", name="t2")
                nc.sync.dma_start(t2[:], t_d[s])

            def res_block(rb, rbi, src, final):
                """src: padded f32 pair.  final=False: return x+conv2(...) bf16 pair;
                final=True: stream x+conv2(...) to DRAM out."""
                a = alloc_padded("act", actp, BF16, zero2b)
                with nc.named_scope(f"s{s}_{rb}_gn1"):
                    gn_relu(src, a, rb, 1)
                h = alloc_padded("pad", padp, F32, zero2)
                with nc.named_scope(f"s{s}_{rb}_conv1"):
                    def eat1(ocb, ls, ps):
                        nc.vector.tensor_scalar_add(
                            h[ocb][:, 1 + ls * LS : 1 + (ls + 1) * LS], ps[:],
                            t2[:, ocb, rbi : rbi + 1],
                        )
                    conv3(a, w1_sb[rb], eat1)
                a2 = alloc_padded("act", actp, BF16, zero2b)
                with nc.named_scope(f"s{s}_{rb}_gn2"):
                    gn_relu(h, a2, rb, 2)
                res = None
                if not final:
                    res = [seqp.tile([P, L], BF16, tag=f"res{cb}", name=f"res{cb}") for cb in range(CB)]
                with nc.named_scope(f"s{s}_{rb}_conv2"):
                    def eat2(ocb, ls, ps):
                        if rb in c2b_sb:
                            nc.vector.tensor_scalar_add(ps[:], ps[:], c2b_sb[rb][:, ocb : ocb + 1])
                        resid = src[ocb][:, 1 + ls * LS : 1 + (ls + 1) * LS]
                        if final:
                            ot = outp.tile([P, LS], F32, tag="out", name="ot")
                            nc.vector.tensor_tensor(out=ot[:], in0=ps[:], in1=resid, op=OP.add)
                            nc.sync.dma_start(
                                out_d[s, ocb * P : (ocb + 1) * P, ls * LS : (ls + 1) * LS], ot[:]
                            )
                        else:
                            nc.vector.tensor_tensor(
                                out=res[ocb][:, ls * LS : (ls + 1) * LS],
                                in0=ps[:], in1=resid, op=OP.add,
                            )
                    conv3(a2, w2_sb[rb], eat2)
                return res

            x1 = res_block("r1", 0, xp, final=False)

            # ---- kqv projections (lin_w rows: [k | q | v]) ----
            kt = [seqp.tile([P, L], BF16, tag=f"kt{cb}", name=f"kt{cb}") for cb in range(CB)]
            qt = [seqp.tile([P, L], BF16, tag=f"qt{cb}", name=f"qt{cb}") for cb in range(CB)]
            # vT with a ones column: [l(P), kb, 0:256]=v, [:, :, 256]=1
            vt = vtp.tile([P, KB, C + 1], BF16, tag="vt", name="vt")
            with nc.named_scope(f"s{s}_kqv"):
                nc.vector.memset(vt[:, :, C : C + 1], 1.0)
                for j, dst, eng in ((0, kt, nc.vector), (1, qt, nc.vector)):
                    for ocb in range(CB):
                        off = j * C + ocb * P
                        for ls in range(NL):
                            ps = pacc.tile([P, LS], F32, tag="acc", name="acc")
                            for icb in range(CB):
                                nc.tensor.matmul(
                                    ps[:],
                                    wkqv_sb[:, icb, off : off + P],
                                    x1[icb][:, ls * LS : (ls + 1) * LS],
                                    start=(icb == 0),
                                    stop=(icb == 1),
                                )
                            dsl = dst[ocb][:, ls * LS : (ls + 1) * LS]
                            if linb_sb is not None:
                                eng.tensor_scalar_add(
                                    dsl, ps[:], linb_sb[:, j * CB + ocb : j * CB + ocb + 1]
                                )
                            else:
                                eng.tensor_copy(out=dsl, in_=ps[:])
                # vT[l, c] (l on partitions) for the attention output matmul
                for lb in range(KB):
                    ps = pacc.tile([P, LS], F32, tag="acc", name="acc")
                    for icb in range(CB):
                        nc.tensor.matmul(
                            ps[:, :C],
                            x1[icb][:, lb * P : (lb + 1) * P],
                            wkqv_sb[:, icb, 2 * C : 3 * C],
                            start=(icb == 0),
                            stop=(icb == 1),
                        )
                    if linb_sb is not None:
                        for cb in range(CB):
                            nc.vector.tensor_scalar_add(
                                vt[:, lb, cb * P : (cb + 1) * P], ps[:, cb * P : (cb + 1) * P],
                                linb_sb[:, 2 * CB + cb : 2 * CB + cb + 1],
                            )
                    else:
                        nc.vector.tensor_copy(out=vt[:, lb, :C], in_=ps[:, :C])

            # ---- attention: softmax over k (no max-subtract; scores are O(8)) ----
            # scores[k, q] per (kb, qs) -> exp -> av^T[q, 0:256] + denom[q] at col 256
            av = alloc_padded("pad", padp, F32, zero2)
            exs = {}
            rdp = {}

            def attn_scores(qs):
                ex = expp.tile([P, KB, LS], BF16, tag="exp", name="exp")
                for kb in range(KB):
                    ps = pacc.tile([P, LS], F32, tag="acc", name="acc")
                    for cb in range(CB):
                        nc.tensor.matmul(
                            ps[:],
                            kt[cb][:, kb * P : (kb + 1) * P],
                            qt[cb][:, qs * LS : (qs + 1) * LS],
                            start=(cb == 0),
                            stop=(cb == 1),
                        )
                    nc.scalar.activation(out=ex[:, kb, :], in_=ps[:], func=AF.Exp, scale=SCALE)
                exs[qs] = ex

            def attn_av(qs):
                ex = exs.pop(qs)
                tr = [ptr.tile([P, LS], F32, tag=f"tr{cb}", name=f"tr{cb}") for cb in range(CB)]
                for qb in range(QB):
                    avt = pacc.tile([P, LS], F32, tag="acc", name="acc")
                    for kb in range(KB):
                        nc.tensor.matmul(
                            avt[:, : C + 1],
                            ex[:, kb, qb * P : (qb + 1) * P],
                            vt[:, kb, :],
                            start=(kb == 0),
                            stop=(kb == KB - 1),
                        )
                    rd = small.tile([P, 1], F32, tag="rd", name="rd")
                    nc.vector.reciprocal(rd[:], avt[:, C : C + 1])
                    avn = avnp.tile([P, C], BF16, tag="avn", name="avn")
                    nc.scalar.activation(out=avn[:], in_=avt[:, :C], func=AF.Copy, scale=rd[:])
                    for cb in range(CB):
                        nc.tensor.matmul(
                            tr[cb][:, qb * P : (qb + 1) * P],
                            avn[:, cb * P : (cb + 1) * P],
                            ident_sb[:],
                            start=True,
                            stop=True,
                        )
                for cb in range(CB):
                    nc.vector.tensor_copy(
                        out=av[cb][:, 1 + qs * LS : 1 + (qs + 1) * LS], in_=tr[cb][:]
                    )

            # software-pipeline: scores(qs+1) issues on PE before av^T(qs) so the
            # PE has work while the exp stream for qs drains on ScalarE
            with nc.named_scope(f"s{s}_attn_sc0"):
                attn_scores(0)
            for qs in range(1, NL):
                with nc.named_scope(f"s{s}_attn_sc{qs}"):
                    attn_scores(qs)
                with nc.named_scope(f"s{s}_attn_av{qs - 1}"):
                    attn_av(qs - 1)
            with nc.named_scope(f"s{s}_attn_av{NL - 1}"):
                attn_av(NL - 1)

            res_block("r2", 1, av, final=True)

    nc.finalize()
    return nc


def _pack_conv_w(w):
    """(O, I, 3) -> [P, icb, tap, oc] bf16."""
    w = np.asarray(w, dtype=np.float32)
    o, i, k = w.shape
    r = np.ascontiguousarray(w.transpose(1, 2, 0).reshape(CB, P, 3, o).transpose(1, 0, 2, 3))
    return r.astype(_mld.bfloat16)


def _pack_gn(v):
    """(256,) -> [P, CB]"""
    return np.ascontiguousarray(np.asarray(v, dtype=np.float32).reshape(CB, P).T)


def make_in_maps(inp, use_bias):
    """Host-side packing; returns the per-core input maps."""
    gind = np.zeros((P, GPB), np.float32)
    bind = np.zeros((CB, P, P), np.float32)
    for cc in range(P):
        gind[cc, cc // 8] = 0.125
        for cb in range(CB):
            bind[cb, cb * 32 + cc // 8, cc] = 1.0
    shared = {
        "wkqvt": np.ascontiguousarray(
            inp["lin_w"][:, :, 0].T.reshape(CB, P, 3 * C).transpose(1, 0, 2)
        ).astype(_mld.bfloat16),
        "gind": gind,
        "bind": bind,
        "ident": np.eye(P, dtype=_mld.bfloat16),
    }
    for rb in ("r1", "r2"):
        shared[f"{rb}_w1t"] = _pack_conv_w(inp[f"{rb}_c1_w"])
        shared[f"{rb}_w2t"] = _pack_conv_w(inp[f"{rb}_c2_w"])
        for ln in (1, 2):
            shared[f"{rb}_gn{ln}_ws"] = _pack_gn(inp[f"{rb}_gn{ln}_w"])
            shared[f"{rb}_gn{ln}_bs"] = _pack_gn(inp[f"{rb}_gn{ln}_b"])
    if "c2b_r1" in use_bias:
        shared["r1_c2bs"] = _pack_gn(inp["r1_c2_b"])
    if "c2b_r2" in use_bias:
        shared["r2_c2bs"] = _pack_gn(inp["r2_c2_b"])
    if "linb" in use_bias:
        shared["lin_bs"] = np.ascontiguousarray(inp["lin_b"].reshape(3 * CB, P).T)

    # per-sample conv1 bias vector: t[s] + c1_b per res block -> [P, CB, 2]
    tfull = inp["t"][:, :, 0]  # (B, C)
    nb = inp["x"].shape[0]
    tv = np.empty((nb, P, CB, 2), np.float32)
    for rbi, rb in enumerate(("r1", "r2")):
        v = tfull + inp[f"{rb}_c1_b"][None, :]
        tv[:, :, :, rbi] = v.reshape(nb, CB, P).transpose(0, 2, 1)

    in_maps = []
    for c in range(NCORES):
        sl = slice(S * c, S * (c + 1))
        m = dict(shared)
        m["x"] = inp["x"][sl]
        m["tv"] = np.ascontiguousarray(tv[sl])
        in_maps.append(m)
    return in_maps


_CACHE = {}


def kernel(**inputs):
    inp = {k: np.ascontiguousarray(np.asarray(v, dtype=np.float32)) for k, v in inputs.items()}

    use_bias = []
    if np.any(inp["r1_c2_b"]):
        use_bias.append("c2b_r1")
    if np.any(inp["r2_c2_b"]):
        use_bias.append("c2b_r2")
    if np.any(inp["lin_b"]):
        use_bias.append("linb")
    use_bias = tuple(use_bias)

    if ("nc", use_bias) not in _CACHE:
        _CACHE[("nc", use_bias)] = build_program(S, use_bias)
    nc = _CACHE[("nc", use_bias)]

    in_maps = make_in_maps(inp, use_bias)
    res = _bu.run_bass_kernel_spmd(nc, in_maps, core_ids=list(range(NCORES)))
    out = np.concatenate([res.results[c]["out"] for c in range(NCORES)], axis=0)
    return out.astype(np.float32)


# revision 9
# speedup vs baseline: 1.3840x; 1.1045x over previous
"""Trainium2 Bass kernel for nn_MidAttnBlock (res-block -> full LxL attention -> res-block).

Contract: kernel(**inputs) takes the FULL inputs of reference.setup_inputs()
(x: (16,256,2048) f32, t: (16,256,1) f32, plus conv/groupnorm/linear params)
and returns the FULL (16,256,2048) f32 output.  Data-parallel over batch on
8 NeuronCores, 2 samples per core; each core runs an identical Bass program.

Performance structure:
- All heavy matmuls run in bf16/fp8: the PE streams the moving operand at
  2 bytes/cycle/partition, so f32r would run at half rate.
- The q.k score matmuls run in fp8e4 DoubleRow (both channel blocks
  contracted in one pass); exp outputs fp8 with a -4.5 shift (softmax is
  shift-invariant, keeps exp within fp8 range).
- av^T = ex^T @ [v | 1] with exp stationary: the softmax denominator falls
  out as column 256 (no ones-matmul), normalize is a per-partition scale
  fused into the PSUM->SBUF copy, transpose back is an N=128 identity matmul.
- The two samples' stages are interleaved so one sample's convs keep the PE
  busy through the other sample's groupnorm reductions (which are
  DVE/ScalarE-only and otherwise stall the PE and re-trigger HAM throttle).

Self-contained: all shapes/sharding hardcoded.
"""

import json as _json

import ml_dtypes as _mld
import numpy as np

import concourse.bass as bass
import concourse.bass2jax as _b2j
import concourse.bass_utils as _bu
import concourse.tile as tile
from concourse import mybir
from concourse.vector_clock import ScopedClock, VectorClock


def _split_bir_waits(bir_json):
    """The walrus_driver in this container encodes at most ONE sync-wait per
    instruction (and none on Drain).  Tile's sem assigner attaches several.
    Rewrite the BIR: excess waits move to single-wait NoOps inserted directly
    before the instruction on the same engine."""
    m = _json.loads(bir_json)
    ctr = 0
    for fn in m.get("functions", []):
        for bb in fn.get("blocks", []):
            out = []
            for ins in bb.get("instructions", []):
                si = ins.get("sync_info")
                waits = (si or {}).get("on_wait") or []
                keep = 0 if ins.get("opcode") == "Drain" else 1
                if len(waits) > keep:
                    nmove = len(waits) - keep
                    for w in waits[:nmove]:
                        ctr += 1
                        out.append({
                            "debug": ins.get("debug", 0),
                            "engine": ins["engine"],
                            "ins": [],
                            "name": f"{ins['name']}-wsp{ctr}",
                            "opcode": "NoOp",
                            "outs": [],
                            "sync_info": {"on_update": [], "on_wait": [w]},
                        })
                    si["on_wait"] = waits[nmove:]
                out.append(ins)
            bb["instructions"] = out
    return _json.dumps(m).encode()


_orig_compile_bir_kernel = _bu.compile_bir_kernel


def _compile_bir_splitwaits(bir_json, tmpdir, neff_name="file.neff"):
    return _orig_compile_bir_kernel(_split_bir_waits(bir_json), tmpdir, neff_name)


if getattr(_bu.compile_bir_kernel, "__name__", "") != "_compile_bir_splitwaits":
    _bu.compile_bir_kernel = _compile_bir_splitwaits
    _b2j.compile_bir_kernel = _compile_bir_splitwaits


F32 = mybir.dt.float32
F32R = mybir.dt.float32r
BF16 = mybir.dt.bfloat16
F8E4 = mybir.dt.float8e4
AF = mybir.ActivationFunctionType
OP = mybir.AluOpType
DRMODE = mybir.MatmulPerfMode.DoubleRow

P = 128          # partitions
C = 256          # channels
CB = 2           # channel blocks of 128
L = 2048         # sequence length
LS = 512         # l-slice (matmul moving dim)
NL = L // LS     # 4 slices
AC = 1024        # gn-apply chunk
NA = L // AC     # 2 apply chunks
KB = L // P      # 16 k-blocks for attention
QB = LS // P     # 4 q-blocks per l-slice
VP = C + 1       # v row width incl. ones column
GPB = 16         # groups per channel-block (32 groups, 8 ch each)
EPS = 1e-5
S = 2            # samples per core
NCORES = 8
SCALE = 1.0 / 16.0  # 1/sqrt(C)
ESHIFT = 4.5     # exp shift so fp8 exp stays in range (softmax invariant)
USE_DR = True    # DoubleRow fp8 for the score matmuls


class _TileContextPatched(tile.TileContext):
    """TileContext whose kernel-tail drain carries no sem waits (the container
    walrus rejects waits on Drain); one SP NOP per proc carries them instead."""

    def _drain_and_barrier(self, tick_clock, wait_clock):
        gc = tick_clock.global_clock
        n = len(gc)
        for p in range(n):
            v = gc[p]
            if v > 0:
                vec = [0] * n
                vec[p] = v
                nop = self.nc.sync.nop()
                wait_clock.add_sem_waits(nop.ins, ScopedClock({None: VectorClock(vec)}))
        self.nc.sync.drain()
        self.nc.all_engine_barrier()
        assert self.sems is not None
        popped = self.nc._tile_sem_poison_stack.pop()
        assert popped is self._sem_poison
        self.nc.clear_and_free_semaphores(list(self.sems.allocated().values()))
        self.nc.all_engine_barrier()


def build_program(samples=S, use_bias=()):
    """Build the per-core Bass program (identical on all cores)."""
    nc = bass.Bass()

    # ---- DRAM I/O (per core) ----
    x_d = nc.dram_tensor("x", (samples, C, L), F32, kind="ExternalInput")
    t_d = nc.dram_tensor("tv", (samples, P, CB, 2), F32, kind="ExternalInput")
    w_conv = {}
    for rb in ("r1", "r2"):
        # host-packed [P(ic within block), icb, tap, oc]
        w_conv[rb, 1] = nc.dram_tensor(f"{rb}_w1t", (P, CB, 3, C), BF16, kind="ExternalInput")
        w_conv[rb, 2] = nc.dram_tensor(f"{rb}_w2t", (P, CB, 3, C), BF16, kind="ExternalInput")
    wkqv_d = nc.dram_tensor("wkqvt", (P, CB, 3 * C), BF16, kind="ExternalInput")
    gnw_d = {}
    for rb in ("r1", "r2"):
        for ln in (1, 2):
            gnw_d[rb, ln, "w"] = nc.dram_tensor(f"{rb}_gn{ln}_ws", (P, CB), F32, kind="ExternalInput")
            gnw_d[rb, ln, "b"] = nc.dram_tensor(f"{rb}_gn{ln}_bs", (P, CB), F32, kind="ExternalInput")
    c2b_d = {}
    if "c2b_r1" in use_bias:
        c2b_d["r1"] = nc.dram_tensor("r1_c2bs", (P, CB), F32, kind="ExternalInput")
    if "c2b_r2" in use_bias:
        c2b_d["r2"] = nc.dram_tensor("r2_c2bs", (P, CB), F32, kind="ExternalInput")
    linb_d = None
    if "linb" in use_bias:
        linb_d = nc.dram_tensor("lin_bs", (P, 3 * CB), F32, kind="ExternalInput")
    gind_d = nc.dram_tensor("gind", (P, GPB), F32R, kind="ExternalInput")  # 1/8 group indicator
    bind_d = nc.dram_tensor("bind", (CB, P, P), F32R, kind="ExternalInput")    # group->channel broadcast
    ident_d = nc.dram_tensor("ident", (P, P), BF16, kind="ExternalInput")
    out_d = nc.dram_tensor("out", (samples, C, L), F32, kind="ExternalOutput")

    with _TileContextPatched(nc) as tc, \
         tc.tile_pool(name="consts", bufs=1) as consts, \
         tc.tile_pool(name="padp", bufs=2) as padp, \
         tc.tile_pool(name="hpad", bufs=4) as hpad, \
         tc.tile_pool(name="actp", bufs=3) as actp, \
         tc.tile_pool(name="seqp", bufs=2) as seqp, \
         tc.tile_pool(name="kqp", bufs=2) as kqp, \
         tc.tile_pool(name="vtp", bufs=2) as vtp, \
         tc.tile_pool(name="expp", bufs=2) as expp, \
         tc.tile_pool(name="outp", bufs=2) as outp, \
         tc.tile_pool(name="avnp", bufs=4) as avnp, \
         tc.tile_pool(name="small", bufs=4) as small, \
         tc.tile_pool(name="t2p", bufs=2) as t2p, \
         tc.tile_pool(name="scp", bufs=2, space="PSUM") as scp, \
         tc.tile_pool(name="pacc", bufs=2, space="PSUM") as pacc, \
         tc.tile_pool(name="ptrp", bufs=1, space="PSUM") as ptrp:

        # ---- persistent constants / weights in SBUF ----
        w1_sb = {}
        w2_sb = {}
        for rb in ("r1", "r2"):
            w1_sb[rb] = consts.tile([P, CB, 3, C], BF16, tag=f"w1_{rb}", name=f"w1_{rb}")
            nc.scalar.dma_start(w1_sb[rb][:], w_conv[rb, 1][:])
            w2_sb[rb] = consts.tile([P, CB, 3, C], BF16, tag=f"w2_{rb}", name=f"w2_{rb}")
            nc.gpsimd.dma_start(w2_sb[rb][:], w_conv[rb, 2][:])
        wkqv_sb = consts.tile([P, CB, 3 * C], BF16, tag="wkqv", name="wkqv")
        nc.scalar.dma_start(wkqv_sb[:], wkqv_d[:])
        gnp_sb = {}
        for rb in ("r1", "r2"):
            for ln in (1, 2):
                for wb in ("w", "b"):
                    tl = consts.tile([P, CB], F32, tag=f"gn_{rb}{ln}{wb}", name=f"gn_{rb}{ln}{wb}")
                    nc.gpsimd.dma_start(tl[:], gnw_d[rb, ln, wb][:])
                    gnp_sb[rb, ln, wb] = tl
        c2b_sb = {}
        for rb, d in c2b_d.items():
            c2b_sb[rb] = consts.tile([P, CB], F32, tag=f"c2b_{rb}", name=f"c2b_{rb}")
            nc.gpsimd.dma_start(c2b_sb[rb][:], d[:])
        linb_sb = None
        if linb_d is not None:
            linb_sb = consts.tile([P, 3 * CB], F32, tag="linb", name="linb")
            nc.gpsimd.dma_start(linb_sb[:], linb_d[:])
        gind_sb = consts.tile([P, GPB], F32R, tag="gind", name="gind")
        nc.gpsimd.dma_start(gind_sb[:], gind_d[:])
        bind_sb = consts.tile([P, CB, P], F32R, tag="bind", name="bind")
        nc.gpsimd.dma_start(bind_sb[:], bind_d.rearrange("cb p c -> p cb c"))
        ident_sb = consts.tile([P, P], BF16, tag="ident", name="ident")
        nc.scalar.dma_start(ident_sb[:], ident_d[:])
        eps_sb = consts.tile([P, 1], F32, tag="eps", name="eps")
        nc.vector.memset(eps_sb[:], EPS)
        zero2 = consts.tile([P, 2], F32, tag="zero2", name="zero2")
        nc.vector.memset(zero2[:], 0.0)
        zero2b = consts.tile([P, 2], BF16, tag="zero2b", name="zero2b")
        nc.vector.memset(zero2b[:], 0.0)
        eshift_sb = consts.tile([P, 1], F32, tag="eshift", name="eshift")
        nc.vector.memset(eshift_sb[:], -ESHIFT)

        def alloc_padded(tag, pool, dt, zt):
            """[P, L+2] tile per channel block; data cols [1, L+1), zero edges."""
            ts = []
            for cb in range(CB):
                tl = pool.tile([P, L + 2], dt, tag=f"{tag}{cb}", name=f"{tag}{cb}")
                nc.vector.tensor_copy(out=tl[:, 0:1], in_=zt[:, 0:1])
                nc.vector.tensor_copy(out=tl[:, L + 1 : L + 2], in_=zt[:, 0:1])
                ts.append(tl)
            return ts

        def gn_stats(src):
            """bn_stats chunks for a padded src pair -> per-cb stats tiles."""
            sts = []
            for cb in range(CB):
                stats = small.tile([P, NL, 6], F32, tag="stats", name="stats")
                for i in range(NL):
                    nc.vector.bn_stats(out=stats[:, i, :], in_=src[cb][:, 1 + i * LS : 1 + (i + 1) * LS])
                sts.append(stats)
            return sts

        def gn_rest(sts, src, dst, rb, ln):
            """Aggregate group stats, then relu((x-m)*rstd*w+b) -> dst (bf16)."""
            gp = []
            for cb in range(CB):
                mv = small.tile([P, 2], F32, tag="mv", name="mv")
                nc.vector.bn_aggr(out=mv[:], in_=sts[cb][:])
                # tmp = [mean_c, E[x^2]_c]  (f32r: feeds the aggregation matmul)
                tmp = small.tile([P, 2], F32R, tag="tmp", name="tmp")
                nc.vector.tensor_copy(out=tmp[:, 0:1], in_=mv[:, 0:1])
                nc.vector.tensor_tensor(out=tmp[:, 1:2], in0=mv[:, 0:1], in1=mv[:, 0:1], op=OP.mult)
                nc.vector.tensor_tensor(out=tmp[:, 1:2], in0=tmp[:, 1:2].bitcast(F32), in1=mv[:, 1:2], op=OP.add)
                g = pacc.tile([P, LS], F32, tag="acc", name="gp")
                nc.tensor.matmul(g[:GPB, 0:2], gind_sb[:], tmp[:], start=True, stop=True)
                gp.append(g)
            # merged group stats; block-cb groups live at partition offset 32*cb
            NG = 32 * CB
            gs = small.tile([NG, 2], F32, tag="gs", name="gs")
            nc.vector.tensor_copy(out=gs[:], in_=zero2[:NG])
            for cb in range(CB):
                nc.vector.tensor_copy(out=gs[cb * 32 : cb * 32 + GPB, :], in_=gp[cb][:GPB, 0:2])
            var = small.tile([NG, 1], F32, tag="var", name="var")
            nc.vector.tensor_tensor(out=var[:], in0=gs[:, 0:1], in1=gs[:, 0:1], op=OP.mult)
            nc.vector.tensor_tensor(out=var[:], in0=gs[:, 1:2], in1=var[:], op=OP.subtract)
            nc.scalar.activation(out=var[:], in_=var[:], func=AF.Ln, bias=eps_sb[:NG])
            rstd = small.tile([NG, 1], F32, tag="rstd", name="rstd")
            nc.scalar.activation(out=rstd[:], in_=var[:], func=AF.Exp, scale=-0.5)
            # pack [rstd_g, -m_g], zero-extended to 128 partitions
            gpk = small.tile([P, 2], F32R, tag="gpk", name="gpk")
            nc.vector.tensor_copy(out=gpk[:], in_=zero2[:])
            nc.vector.tensor_copy(out=gpk[:NG, 0:1], in_=rstd[:])
            nc.vector.tensor_scalar_mul(gpk[:NG, 1:2], gs[:, 0:1], -1.0)
            for cb in range(CB):
                # broadcast to channels: bc[c, :] = [rstd_g(c), -m_g(c)]
                bc = pacc.tile([P, LS], F32, tag="acc", name="bc")
                nc.tensor.matmul(bc[:, 0:2], bind_sb[:, cb, :], gpk[:], start=True, stop=True)
                sb = small.tile([P, 2], F32, tag="sb", name="sb")
                # s = rstd*w ; b2 = b - m*s
                nc.vector.tensor_scalar_mul(sb[:, 0:1], bc[:, 0:1], gnp_sb[rb, ln, "w"][:, cb : cb + 1])
                nc.vector.tensor_tensor(out=sb[:, 1:2], in0=bc[:, 1:2], in1=sb[:, 0:1], op=OP.mult)
                nc.vector.tensor_scalar_add(sb[:, 1:2], sb[:, 1:2], gnp_sb[rb, ln, "b"][:, cb : cb + 1])
                # apply + relu on ScalarE in NA chunks so convs can start early
                for i in range(NA):
                    nc.scalar.activation(
                        out=dst[cb][:, 1 + i * AC : 1 + (i + 1) * AC],
                        in_=src[cb][:, 1 + i * AC : 1 + (i + 1) * AC],
                        func=AF.Relu,
                        bias=sb[:, 1:2],
                        scale=sb[:, 0:1],
                    )

        def conv3(src, wt, consume):
            """3-tap conv over padded bf16 src; consume(ocb, ls, psum_tile)."""
            for ocb in range(CB):
                for ls in range(NL):
                    ps = pacc.tile([P, LS], F32, tag="acc", name="acc")
                    k = 0
                    for icb in range(CB):
                        for tap in range(3):
                            nc.tensor.matmul(
                                ps[:],
                                wt[:, icb, tap, ocb * P : (ocb + 1) * P],
                                src[icb][:, ls * LS + tap : ls * LS + tap + LS],
                                start=(k == 0),
                                stop=(k == 5),
                            )
                            k += 1
                    consume(ocb, ls, ps)

        # ---- per-sample staged pipeline ----
        st = [dict() for _ in range(samples)]

        def stg_load(s):
            with nc.named_scope(f"s{s}_load"):
                xp = alloc_padded("pad", padp, F32, zero2)
                for cb in range(CB):
                    for i in range(NL):
                        nc.sync.dma_start(
                            xp[cb][:, 1 + i * LS : 1 + (i + 1) * LS],
                            x_d[s, cb * P : (cb + 1) * P, i * LS : (i + 1) * LS],
                        )
                t2 = t2p.tile([P, CB, 2], F32, tag="t2", name="t2")
                nc.sync.dma_start(t2[:], t_d[s])
                st[s]["xp"] = xp
                st[s]["t2"] = t2

        def stg_gn1s(s, rb):
            src = st[s]["xp"] if rb == "r1" else st[s]["av"]
            with nc.named_scope(f"s{s}_{rb}_gn1s"):
                st[s]["gn1st"] = gn_stats(src)

        def stg_gn1r(s, rb):
            src = st[s]["xp"] if rb == "r1" else st[s]["av"]
            with nc.named_scope(f"s{s}_{rb}_gn1r"):
                a = alloc_padded("act", actp, BF16, zero2b)
                gn_rest(st[s].pop("gn1st"), src, a, rb, 1)
                st[s]["a"] = a

        def stg_conv1(s, rb, rbi):
            t2 = st[s]["t2"]
            with nc.named_scope(f"s{s}_{rb}_conv1"):
                h = alloc_padded("hp", hpad, BF16, zero2b)

                def eat1(ocb, ls, ps):
                    nc.vector.tensor_scalar_add(
                        h[ocb][:, 1 + ls * LS : 1 + (ls + 1) * LS], ps[:],
                        t2[:, ocb, rbi : rbi + 1],
                    )
                conv3(st[s].pop("a"), w1_sb[rb], eat1)
                st[s]["h"] = h

        def stg_gn2s(s, rb):
            with nc.named_scope(f"s{s}_{rb}_gn2s"):
                st[s]["gn2st"] = gn_stats(st[s]["h"])

        def stg_gn2r(s, rb):
            with nc.named_scope(f"s{s}_{rb}_gn2r"):
                a2 = alloc_padded("act", actp, BF16, zero2b)
                gn_rest(st[s].pop("gn2st"), st[s]["h"], a2, rb, 2)
                st[s]["a2"] = a2

        def stg_conv2(s, rb):
            final = rb == "r2"
            src = st[s]["xp"] if rb == "r1" else st[s]["av"]
            with nc.named_scope(f"s{s}_{rb}_conv2"):
                res = None
                if not final:
                    res = [seqp.tile([P, L], BF16, tag=f"res{cb}", name=f"res{cb}") for cb in range(CB)]

                def eat2(ocb, ls, ps):
                    if rb in c2b_sb:
                        nc.vector.tensor_scalar_add(ps[:], ps[:], c2b_sb[rb][:, ocb : ocb + 1])
                    resid = src[ocb][:, 1 + ls * LS : 1 + (ls + 1) * LS]
                    if final:
                        ot = outp.tile([P, LS], F32, tag="out", name="ot")
                        nc.vector.tensor_tensor(out=ot[:], in0=ps[:], in1=resid, op=OP.add)
                        nc.sync.dma_start(
                            out_d[s, ocb * P : (ocb + 1) * P, ls * LS : (ls + 1) * LS], ot[:]
                        )
                    else:
                        nc.vector.tensor_tensor(
                            out=res[ocb][:, ls * LS : (ls + 1) * LS],
                            in0=ps[:], in1=resid, op=OP.add,
                        )
                conv3(st[s].pop("a2"), w2_sb[rb], eat2)
                if not final:
                    st[s]["x1"] = res

        def stg_kqv(s):
            """kqv projections (lin_w rows: [k | q | v]); k/q packed fp8
            [P(c within block), cb, L] for the DoubleRow score matmuls."""
            x1 = st[s].pop("x1")
            kq = {}
            with nc.named_scope(f"s{s}_kqv"):
                for j, nm in ((0, "kt"), (1, "qt")):
                    dst = kqp.tile([P, CB, L], F8E4, tag=nm, name=nm)
                    kq[nm] = dst
                    for ocb in range(CB):
                        off = j * C + ocb * P
                        for ls in range(NL):
                            ps = pacc.tile([P, LS], F32, tag="acc", name="acc")
                            for icb in range(CB):
                                nc.tensor.matmul(
                                    ps[:],
                                    wkqv_sb[:, icb, off : off + P],
                                    x1[icb][:, ls * LS : (ls + 1) * LS],
                                    start=(icb == 0),
                                    stop=(icb == 1),
                                )
                            dsl = dst[:, ocb, ls * LS : (ls + 1) * LS]
                            if linb_sb is not None:
                                nc.vector.tensor_scalar_add(
                                    dsl, ps[:], linb_sb[:, j * CB + ocb : j * CB + ocb + 1]
                                )
                            else:
                                nc.vector.tensor_copy(out=dsl, in_=ps[:])
                # vT[l, c] (l on partitions) with ones col for the denominator
                vt = vtp.tile([P, KB, VP], F8E4, tag="vt", name="vt")
                nc.vector.memset(vt[:, :, C : C + 1], 1.0)
                for lb in range(KB):
                    ps = pacc.tile([P, LS], F32, tag="acc", name="acc")
                    for icb in range(CB):
                        nc.tensor.matmul(
                            ps[:, :C],
                            x1[icb][:, lb * P : (lb + 1) * P],
                            wkqv_sb[:, icb, 2 * C : 3 * C],
                            start=(icb == 0),
                            stop=(icb == 1),
                        )
                    if linb_sb is not None:
                        for cb in range(CB):
                            nc.vector.tensor_scalar_add(
                                vt[:, lb, cb * P : (cb + 1) * P], ps[:, cb * P : (cb + 1) * P],
                                linb_sb[:, 2 * CB + cb : 2 * CB + cb + 1],
                            )
                    else:
                        nc.vector.tensor_copy(out=vt[:, lb, :C], in_=ps[:, :C])
                st[s]["kt"] = kq["kt"]
                st[s]["qt"] = kq["qt"]
                st[s]["vt"] = vt

        def stg_attn(s):
            """softmax over k: scores[k,q] -> exp (fp8, shifted) ->
            av^T[q, 0:256] + denom[q] at col 256 -> normalize -> transpose."""
            kt, qt, vt = st[s]["kt"], st[s]["qt"], st[s]["vt"]
            av = alloc_padded("hp", hpad, BF16, zero2b)
            exs = {}

            def attn_scores(qs):
                ex = expp.tile([P, KB, LS], F8E4, tag="exp", name="exp")
                for kp in range(KB // 2):
                    ps = scp.tile([P, 2, LS], F32, tag="sc", name="sc")
                    for j in range(2):
                        kb = 2 * kp + j
                        if USE_DR:
                            nc.tensor.matmul(
                                ps[:, j, :],
                                kt[:, :, kb * P : (kb + 1) * P],
                                qt[:, :, qs * LS : (qs + 1) * LS],
                                start=True,
                                stop=True,
                                perf_mode=DRMODE,
                            )
                        else:
                            for cb in range(CB):
                                nc.tensor.matmul(
                                    ps[:, j, :],
                                    kt[:, cb, kb * P : (kb + 1) * P],
                                    qt[:, cb, qs * LS : (qs + 1) * LS],
                                    start=(cb == 0),
                                    stop=(cb == 1),
                                )
                    nc.scalar.activation(
                        out=ex[:, 2 * kp : 2 * kp + 2, :], in_=ps[:, :, :],
                        func=AF.Exp, scale=SCALE, bias=eshift_sb[:],
                    )
                exs[qs] = ex

            def attn_av(qs):
                ex = exs.pop(qs)
                tr = ptrp.tile([P, CB, LS], F32, tag="tr", name="tr")
                for qb in range(QB):
                    avt = pacc.tile([P, LS], F32, tag="acc", name="avt")
                    for kb in range(KB):
                        nc.tensor.matmul(
                            avt[:, :VP],
                            ex[:, kb, qb * P : (qb + 1) * P],
                            vt[:, kb, :],
                            start=(kb == 0),
                            stop=(kb == KB - 1),
                        )
                    rd = small.tile([P, 1], F32, tag="rd", name="rd")
                    nc.vector.reciprocal(rd[:], avt[:, C : C + 1])
                    avn = avnp.tile([P, C], BF16, tag="avn", name="avn")
                    nc.scalar.activation(out=avn[:], in_=avt[:, :C], func=AF.Copy, scale=rd[:])
                    for cb in range(CB):
                        nc.tensor.matmul(
                            tr[:, cb, qb * P : (qb + 1) * P],
                            avn[:, cb * P : (cb + 1) * P],
                            ident_sb[:],
                            start=True,
                            stop=True,
                        )
                for cb in range(CB):
                    nc.vector.tensor_copy(
                        out=av[cb][:, 1 + qs * LS : 1 + (qs + 1) * LS], in_=tr[:, cb, :]
                    )

            # software-pipeline: scores(qs+1) issues on PE before av^T(qs) so the
            # PE has work while the exp stream for qs drains on ScalarE
            with nc.named_scope(f"s{s}_attn_sc0"):
                attn_scores(0)
            for qs in range(1, NL):
                with nc.named_scope(f"s{s}_attn_sc{qs}"):
                    attn_scores(qs)
                with nc.named_scope(f"s{s}_attn_av{qs - 1}"):
                    attn_av(qs - 1)
            with nc.named_scope(f"s{s}_attn_av{NL - 1}"):
                attn_av(NL - 1)
            st[s]["av"] = av

        # ---- interleaved emission: sample B's convs cover sample A's GNs ----
        assert samples == 2
        A, B = 0, 1
        stg_load(A)
        stg_load(B)
        order = [
            (stg_gn1s, A, "r1"), (stg_gn1r, A, "r1"),
            (stg_gn1s, B, "r1"),
            (stg_conv1, A, "r1", 0),
            (stg_gn1r, B, "r1"),
            (stg_gn2s, A, "r1"),
            (stg_conv1, B, "r1", 0),
            (stg_gn2r, A, "r1"),
            (stg_conv2, A, "r1"),
            (stg_gn2s, B, "r1"), (stg_gn2r, B, "r1"),
            (stg_kqv, A),
            (stg_conv2, B, "r1"),
            (stg_attn, A),
            (stg_gn1s, A, "r2"),
            (stg_kqv, B),
            (stg_gn1r, A, "r2"),
            (stg_attn, B),
            (stg_conv1, A, "r2", 1),
            (stg_gn1s, B, "r2"), (stg_gn1r, B, "r2"),
            (stg_gn2s, A, "r2"),
            (stg_conv1, B, "r2", 1),
            (stg_gn2r, A, "r2"),
            (stg_conv2, A, "r2"),
            (stg_gn2s, B, "r2"), (stg_gn2r, B, "r2"),
            (stg_conv2, B, "r2"),
        ]
        for fn, *args in order:
            fn(*args)

    nc.finalize()
    return nc


def _pack_conv_w(w):
    """(O, I, 3) -> [P, icb, tap, oc] bf16."""
    w = np.asarray(w, dtype=np.float32)
    o, i, k = w.shape
    r = np.ascontiguousarray(w.transpose(1, 2, 0).reshape(CB, P, 3, o).transpose(1, 0, 2, 3))
    return r.astype(_mld.bfloat16)


def _pack_gn(v):
    """(256,) -> [P, CB]"""
    return np.ascontiguousarray(np.asarray(v, dtype=np.float32).reshape(CB, P).T)


def make_in_maps(inp, use_bias):
    """Host-side packing; returns the per-core input maps."""
    gind = np.zeros((P, GPB), np.float32)
    bind = np.zeros((CB, P, P), np.float32)
    for cc in range(P):
        gind[cc, cc // 8] = 0.125
        for cb in range(CB):
            bind[cb, cb * 32 + cc // 8, cc] = 1.0
    shared = {
        "wkqvt": np.ascontiguousarray(
            inp["lin_w"][:, :, 0].T.reshape(CB, P, 3 * C).transpose(1, 0, 2)
        ).astype(_mld.bfloat16),
        "gind": gind,
        "bind": bind,
        "ident": np.eye(P, dtype=_mld.bfloat16),
    }
    for rb in ("r1", "r2"):
        shared[f"{rb}_w1t"] = _pack_conv_w(inp[f"{rb}_c1_w"])
        shared[f"{rb}_w2t"] = _pack_conv_w(inp[f"{rb}_c2_w"])
        for ln in (1, 2):
            shared[f"{rb}_gn{ln}_ws"] = _pack_gn(inp[f"{rb}_gn{ln}_w"])
            shared[f"{rb}_gn{ln}_bs"] = _pack_gn(inp[f"{rb}_gn{ln}_b"])
    if "c2b_r1" in use_bias:
        shared["r1_c2bs"] = _pack_gn(inp["r1_c2_b"])
    if "c2b_r2" in use_bias:
        shared["r2_c2bs"] = _pack_gn(inp["r2_c2_b"])
    if "linb" in use_bias:
        shared["lin_bs"] = np.ascontiguousarray(inp["lin_b"].reshape(3 * CB, P).T)

    # per-sample conv1 bias vector: t[s] + c1_b per res block -> [P, CB, 2]
    tfull = inp["t"][:, :, 0]  # (B, C)
    nb = inp["x"].shape[0]
    tv = np.empty((nb, P, CB, 2), np.float32)
    for rbi, rb in enumerate(("r1", "r2")):
        v = tfull + inp[f"{rb}_c1_b"][None, :]
        tv[:, :, :, rbi] = v.reshape(nb, CB, P).transpose(0, 2, 1)

    in_maps = []
    for c in range(NCORES):
        sl = slice(S * c, S * (c + 1))
        m = dict(shared)
        m["x"] = inp["x"][sl]
        m["tv"] = np.ascontiguousarray(tv[sl])
        in_maps.append(m)
    return in_maps


_CACHE = {}


def kernel(**inputs):
    inp = {k: np.ascontiguousarray(np.asarray(v, dtype=np.float32)) for k, v in inputs.items()}

    use_bias = []
    if np.any(inp["r1_c2_b"]):
        use_bias.append("c2b_r1")
    if np.any(inp["r2_c2_b"]):
        use_bias.append("c2b_r2")
    if np.any(inp["lin_b"]):
        use_bias.append("linb")
    use_bias = tuple(use_bias)

    if ("nc", use_bias) not in _CACHE:
        _CACHE[("nc", use_bias)] = build_program(S, use_bias)
    nc = _CACHE[("nc", use_bias)]

    in_maps = make_in_maps(inp, use_bias)
    res = _bu.run_bass_kernel_spmd(nc, in_maps, core_ids=list(range(NCORES)))
    out = np.concatenate([res.results[c]["out"] for c in range(NCORES)], axis=0)
    return out.astype(np.float32)


# revision 28
# speedup vs baseline: 1.4233x; 1.0284x over previous
"""Trainium2 Bass kernel for nn_MidAttnBlock (res-block -> full LxL attention -> res-block).

Contract: kernel(**inputs) takes the FULL inputs of reference.setup_inputs()
(x: (16,256,2048) f32, t: (16,256,1) f32, plus conv/groupnorm/linear params)
and returns the FULL (16,256,2048) f32 output.  Data-parallel over batch on
8 NeuronCores, 2 samples per core; each core runs an identical Bass program.

Performance structure:
- All heavy matmuls run in bf16/fp8: the PE streams the moving operand at
  2 bytes/cycle/partition, so f32r would run at half rate.
- The q.k score matmuls run in fp8e4 DoubleRow (both channel blocks
  contracted in one pass); exp outputs fp8 with a -4.5 shift (softmax is
  shift-invariant, keeps exp within fp8 range).
- av^T = ex^T @ [v | 1] with exp stationary: the softmax denominator falls
  out as column 256 (no ones-matmul), normalize is a per-partition scale
  fused into the PSUM->SBUF copy, transpose back is an N=128 identity matmul.
- The two samples' stages are interleaved so one sample's convs keep the PE
  busy through the other sample's groupnorm reductions (which are
  DVE/ScalarE-only and otherwise stall the PE and re-trigger HAM throttle).

Self-contained: all shapes/sharding hardcoded.
"""

import json as _json

import ml_dtypes as _mld
import numpy as np

import concourse.bass as bass
import concourse.bass2jax as _b2j
import concourse.bass_utils as _bu
import concourse.tile as tile
from concourse import mybir
from concourse.vector_clock import ScopedClock, VectorClock


def _split_bir_waits(bir_json):
    """The walrus_driver in this container encodes at most ONE sync-wait per
    instruction (and none on Drain).  Tile's sem assigner attaches several.
    Rewrite the BIR: excess waits move to single-wait NoOps inserted directly
    before the instruction on the same engine."""
    m = _json.loads(bir_json)
    ctr = 0
    for fn in m.get("functions", []):
        for bb in fn.get("blocks", []):
            out = []
            for ins in bb.get("instructions", []):
                si = ins.get("sync_info")
                waits = (si or {}).get("on_wait") or []
                keep = 0 if ins.get("opcode") == "Drain" else 1
                if len(waits) > keep:
                    nmove = len(waits) - keep
                    for w in waits[:nmove]:
                        ctr += 1
                        out.append({
                            "debug": ins.get("debug", 0),
                            "engine": ins["engine"],
                            "ins": [],
                            "name": f"{ins['name']}-wsp{ctr}",
                            "opcode": "NoOp",
                            "outs": [],
                            "sync_info": {"on_update": [], "on_wait": [w]},
                        })
                    si["on_wait"] = waits[nmove:]
                out.append(ins)
            bb["instructions"] = out
    return _json.dumps(m).encode()


_orig_compile_bir_kernel = _bu.compile_bir_kernel


def _compile_bir_splitwaits(bir_json, tmpdir, neff_name="file.neff"):
    return _orig_compile_bir_kernel(_split_bir_waits(bir_json), tmpdir, neff_name)


if getattr(_bu.compile_bir_kernel, "__name__", "") != "_compile_bir_splitwaits":
    _bu.compile_bir_kernel = _compile_bir_splitwaits
    _b2j.compile_bir_kernel = _compile_bir_splitwaits


F32 = mybir.dt.float32
F32R = mybir.dt.float32r
BF16 = mybir.dt.bfloat16
F8E4 = mybir.dt.float8e4
AF = mybir.ActivationFunctionType
OP = mybir.AluOpType
DRMODE = mybir.MatmulPerfMode.DoubleRow

P = 128          # partitions
C = 256          # channels
CB = 2           # channel blocks of 128
L = 2048         # sequence length
LS = 512         # l-slice (matmul moving dim)
NL = L // LS     # 4 slices
AC = 1024        # gn-apply chunk
NA = L // AC     # 2 apply chunks
KB = L // P      # 16 k-blocks for attention
QB = LS // P     # 4 q-blocks per l-slice
VP = C + 1       # v row width incl. ones column
VPP = 272        # vt free pitch (16B-aligned for DoubleRow pair stride)
GPB = 16         # groups per channel-block (32 groups, 8 ch each)
EPS = 1e-5
S = 2            # samples per core
NCORES = 8
SCALE = 1.0 / 16.0  # 1/sqrt(C)
ESHIFT = 4.5     # exp shift so fp8 exp stays in range (softmax invariant)
USE_DR = True    # DoubleRow fp8 for the score matmuls
AV_DR = True     # DoubleRow fp8 for the av^T matmuls
DEBUG = False    # dump intermediates to DRAM


class _TileContextPatched(tile.TileContext):
    """TileContext whose kernel-tail drain carries no sem waits (the container
    walrus rejects waits on Drain); one SP NOP per proc carries them instead."""

    def _drain_and_barrier(self, tick_clock, wait_clock):
        gc = tick_clock.global_clock
        n = len(gc)
        for p in range(n):
            v = gc[p]
            if v > 0:
                vec = [0] * n
                vec[p] = v
                nop = self.nc.sync.nop()
                wait_clock.add_sem_waits(nop.ins, ScopedClock({None: VectorClock(vec)}))
        self.nc.sync.drain()
        self.nc.all_engine_barrier()
        assert self.sems is not None
        popped = self.nc._tile_sem_poison_stack.pop()
        assert popped is self._sem_poison
        self.nc.clear_and_free_semaphores(list(self.sems.allocated().values()))
        self.nc.all_engine_barrier()


def build_program(samples=S, use_bias=()):
    """Build the per-core Bass program (identical on all cores)."""
    nc = bass.Bass()

    # ---- DRAM I/O (per core) ----
    x_d = nc.dram_tensor("x", (samples, C, L), F32, kind="ExternalInput")
    t_d = nc.dram_tensor("tv", (samples, P, CB, 2), F32, kind="ExternalInput")
    w_conv = {}
    for rb in ("r1", "r2"):
        # host-packed [P(ic within block), icb, tap, oc]
        w_conv[rb, 1] = nc.dram_tensor(f"{rb}_w1t", (P, CB, 3, C), BF16, kind="ExternalInput")
        w_conv[rb, 2] = nc.dram_tensor(f"{rb}_w2t", (P, CB, 3, C), BF16, kind="ExternalInput")
    wkqv_d = nc.dram_tensor("wkqvt", (P, CB, 3 * C), BF16, kind="ExternalInput")
    gnw_d = {}
    for rb in ("r1", "r2"):
        for ln in (1, 2):
            gnw_d[rb, ln, "w"] = nc.dram_tensor(f"{rb}_gn{ln}_ws", (P, CB), F32, kind="ExternalInput")
            gnw_d[rb, ln, "b"] = nc.dram_tensor(f"{rb}_gn{ln}_bs", (P, CB), F32, kind="ExternalInput")
    c2b_d = {}
    if "c2b_r1" in use_bias:
        c2b_d["r1"] = nc.dram_tensor("r1_c2bs", (P, CB), F32, kind="ExternalInput")
    if "c2b_r2" in use_bias:
        c2b_d["r2"] = nc.dram_tensor("r2_c2bs", (P, CB), F32, kind="ExternalInput")
    linb_d = None
    if "linb" in use_bias:
        linb_d = nc.dram_tensor("lin_bs", (P, 3 * CB), F32, kind="ExternalInput")
    gind_d = nc.dram_tensor("gind", (P, GPB), BF16, kind="ExternalInput")  # 1/8 group indicator
    bind_d = nc.dram_tensor("bind", (CB, P, P), BF16, kind="ExternalInput")    # group->channel broadcast
    ident_d = nc.dram_tensor("ident", (P, P), BF16, kind="ExternalInput")
    out_d = nc.dram_tensor("out", (samples, C, L), F32, kind="ExternalOutput")
    if DEBUG:
        dbga_d = nc.dram_tensor("dbg_a", (P, L), BF16, kind="ExternalOutput")
        dbgh_d = nc.dram_tensor("dbg_h", (P, L), BF16, kind="ExternalOutput")
        dbgvt_d = nc.dram_tensor("dbg_vt", (P, KB, VPP), F8E4, kind="ExternalOutput")
        dbgav_d = nc.dram_tensor("dbg_av", (P, L), BF16, kind="ExternalOutput")

    with _TileContextPatched(nc) as tc, \
         tc.tile_pool(name="consts", bufs=1) as consts, \
         tc.tile_pool(name="padp", bufs=2) as padp, \
         tc.tile_pool(name="hpad", bufs=4) as hpad, \
         tc.tile_pool(name="actp", bufs=3) as actp, \
         tc.tile_pool(name="seqp", bufs=2) as seqp, \
         tc.tile_pool(name="kqp", bufs=2) as kqp, \
         tc.tile_pool(name="vtp", bufs=2) as vtp, \
         tc.tile_pool(name="expp", bufs=2) as expp, \
         tc.tile_pool(name="outp", bufs=2) as outp, \
         tc.tile_pool(name="avnp", bufs=4) as avnp, \
         tc.tile_pool(name="small", bufs=4) as small, \
         tc.tile_pool(name="t2p", bufs=2) as t2p, \
         tc.tile_pool(name="scp", bufs=2, space="PSUM") as scp, \
         tc.tile_pool(name="pacc", bufs=2, space="PSUM") as pacc, \
         tc.tile_pool(name="ptrp", bufs=1, space="PSUM") as ptrp:

        # ---- persistent constants / weights in SBUF ----
        w1_sb = {}
        w2_sb = {}
        for rb in ("r1", "r2"):
            w1_sb[rb] = consts.tile([P, CB, 3, C], BF16, tag=f"w1_{rb}", name=f"w1_{rb}")
            nc.scalar.dma_start(w1_sb[rb][:], w_conv[rb, 1][:])
            w2_sb[rb] = consts.tile([P, CB, 3, C], BF16, tag=f"w2_{rb}", name=f"w2_{rb}")
            nc.gpsimd.dma_start(w2_sb[rb][:], w_conv[rb, 2][:])
        wkqv_sb = consts.tile([P, CB, 3 * C], BF16, tag="wkqv", name="wkqv")
        nc.scalar.dma_start(wkqv_sb[:], wkqv_d[:])
        gnp_sb = {}
        for rb in ("r1", "r2"):
            for ln in (1, 2):
                for wb in ("w", "b"):
                    tl = consts.tile([P, CB], F32, tag=f"gn_{rb}{ln}{wb}", name=f"gn_{rb}{ln}{wb}")
                    nc.gpsimd.dma_start(tl[:], gnw_d[rb, ln, wb][:])
                    gnp_sb[rb, ln, wb] = tl
        c2b_sb = {}
        for rb, d in c2b_d.items():
            c2b_sb[rb] = consts.tile([P, CB], F32, tag=f"c2b_{rb}", name=f"c2b_{rb}")
            nc.gpsimd.dma_start(c2b_sb[rb][:], d[:])
        linb_sb = None
        if linb_d is not None:
            linb_sb = consts.tile([P, 3 * CB], F32, tag="linb", name="linb")
            nc.gpsimd.dma_start(linb_sb[:], linb_d[:])
        gind_sb = consts.tile([P, GPB], BF16, tag="gind", name="gind")
        nc.gpsimd.dma_start(gind_sb[:], gind_d[:])
        bind_sb = consts.tile([P, CB, P], BF16, tag="bind", name="bind")
        nc.gpsimd.dma_start(bind_sb[:], bind_d.rearrange("cb p c -> p cb c"))
        ident_sb = consts.tile([P, P], BF16, tag="ident", name="ident")
        nc.scalar.dma_start(ident_sb[:], ident_d[:])
        eps_sb = consts.tile([P, 1], F32, tag="eps", name="eps")
        nc.vector.memset(eps_sb[:], EPS)
        zero2 = consts.tile([P, 2], F32, tag="zero2", name="zero2")
        nc.vector.memset(zero2[:], 0.0)
        zero2b = consts.tile([P, 2], BF16, tag="zero2b", name="zero2b")
        nc.vector.memset(zero2b[:], 0.0)
        eshift_sb = consts.tile([P, 1], F32, tag="eshift", name="eshift")
        nc.vector.memset(eshift_sb[:], -ESHIFT)

        def alloc_padded(tag, pool, dt, zt):
            """[P, L+2] tile per channel block; data cols [1, L+1), zero edges."""
            ts = []
            for cb in range(CB):
                tl = pool.tile([P, L + 2], dt, tag=f"{tag}{cb}", name=f"{tag}{cb}")
                nc.vector.tensor_copy(out=tl[:, 0:1], in_=zt[:, 0:1])
                nc.vector.tensor_copy(out=tl[:, L + 1 : L + 2], in_=zt[:, 0:1])
                ts.append(tl)
            return ts

        def gn_stats(src):
            """bn_stats chunks for a padded src pair -> per-cb stats tiles."""
            sts = []
            for cb in range(CB):
                stats = small.tile([P, NL, 6], F32, tag="stats", name="stats")
                for i in range(NL):
                    nc.vector.bn_stats(out=stats[:, i, :], in_=src[cb][:, 1 + i * LS : 1 + (i + 1) * LS])
                sts.append(stats)
            return sts

        def gn_rest(sts, src, dst, rb, ln):
            """Aggregate group stats, then relu((x-m)*rstd*w+b) -> dst (bf16).

            Small chain ops are spread over GpSimd/ScalarE so the serial
            latency does not queue behind VectorE's bulk work (which stalls
            the PE at the following conv)."""
            gp = []
            for cb in range(CB):
                mv = small.tile([P, 2], F32, tag="mv", name="mv")
                nc.vector.bn_aggr(out=mv[:], in_=sts[cb][:])
                # tmp = [mean_c, E[x^2]_c]  (bf16: feeds the aggregation matmul)
                tmp = small.tile([P, 2], BF16, tag="tmp", name="tmp")
                nc.vector.tensor_copy(out=tmp[:, 0:1], in_=mv[:, 0:1])
                nc.vector.tensor_tensor(out=tmp[:, 1:2], in0=mv[:, 0:1], in1=mv[:, 0:1], op=OP.mult)
                nc.vector.tensor_tensor(out=tmp[:, 1:2], in0=tmp[:, 1:2], in1=mv[:, 1:2], op=OP.add)
                g = pacc.tile([P, LS], F32, tag="acc", name="gp")
                nc.tensor.matmul(g[:GPB, 0:2], gind_sb[:], tmp[:], start=True, stop=True)
                gp.append(g)
            # merged group stats; block-cb groups live at partition offset 32*cb
            NG = 32 * CB
            gs = small.tile([NG, 2], F32, tag="gs", name="gs")
            nc.vector.tensor_copy(out=gs[:], in_=zero2[:NG])
            for cb in range(CB):
                nc.vector.tensor_copy(out=gs[cb * 32 : cb * 32 + GPB, :], in_=gp[cb][:GPB, 0:2])
            var = small.tile([NG, 1], F32, tag="var", name="var")
            nc.vector.tensor_tensor(out=var[:], in0=gs[:, 0:1], in1=gs[:, 0:1], op=OP.mult)
            nc.vector.tensor_tensor(out=var[:], in0=gs[:, 1:2], in1=var[:], op=OP.subtract)
            nc.scalar.activation(out=var[:], in_=var[:], func=AF.Ln, bias=eps_sb[:NG])
            rstd = small.tile([NG, 1], F32, tag="rstd", name="rstd")
            nc.scalar.activation(out=rstd[:], in_=var[:], func=AF.Exp, scale=-0.5)
            # pack [rstd_g, -m_g], zero-extended to 128 partitions
            gpk = small.tile([P, 2], BF16, tag="gpk", name="gpk")
            nc.vector.tensor_copy(out=gpk[:], in_=zero2[:])
            nc.vector.tensor_copy(out=gpk[:NG, 0:1], in_=rstd[:])
            nc.vector.tensor_scalar_mul(gpk[:NG, 1:2], gs[:, 0:1], -1.0)
            for cb in range(CB):
                # broadcast to channels: bc[c, :] = [rstd_g(c), -m_g(c)]
                bc = pacc.tile([P, LS], F32, tag="acc", name="bc")
                nc.tensor.matmul(bc[:, 0:2], bind_sb[:, cb, :], gpk[:], start=True, stop=True)
                sb = small.tile([P, 2], F32, tag="sb", name="sb")
                # s = rstd*w ; b2 = b - m*s
                nc.vector.tensor_scalar_mul(sb[:, 0:1], bc[:, 0:1], gnp_sb[rb, ln, "w"][:, cb : cb + 1])
                nc.vector.tensor_tensor(out=sb[:, 1:2], in0=bc[:, 1:2], in1=sb[:, 0:1], op=OP.mult)
                nc.vector.tensor_scalar_add(sb[:, 1:2], sb[:, 1:2], gnp_sb[rb, ln, "b"][:, cb : cb + 1])
                # apply + relu on ScalarE in NA chunks so convs can start early
                for i in range(NA):
                    nc.scalar.activation(
                        out=dst[cb][:, 1 + i * AC : 1 + (i + 1) * AC],
                        in_=src[cb][:, 1 + i * AC : 1 + (i + 1) * AC],
                        func=AF.Relu,
                        bias=sb[:, 1:2],
                        scale=sb[:, 0:1],
                    )

        def conv3(src, wt, consume):
            """3-tap conv over padded bf16 src; consume(ocb, ls, psum_tile)."""
            for ocb in range(CB):
                for ls in range(NL):
                    ps = pacc.tile([P, LS], F32, tag="acc", name="acc")
                    k = 0
                    for icb in range(CB):
                        for tap in range(3):
                            nc.tensor.matmul(
                                ps[:],
                                wt[:, icb, tap, ocb * P : (ocb + 1) * P],
                                src[icb][:, ls * LS + tap : ls * LS + tap + LS],
                                start=(k == 0),
                                stop=(k == 5),
                            )
                            k += 1
                    consume(ocb, ls, ps)

        # ---- per-sample staged pipeline ----
        st = [dict() for _ in range(samples)]

        def stg_load(s):
            with nc.named_scope(f"s{s}_load"):
                xp = alloc_padded("pad", padp, F32, zero2)
                for cb in range(CB):
                    for i in range(NL):
                        nc.sync.dma_start(
                            xp[cb][:, 1 + i * LS : 1 + (i + 1) * LS],
                            x_d[s, cb * P : (cb + 1) * P, i * LS : (i + 1) * LS],
                        )
                t2 = t2p.tile([P, CB, 2], F32, tag="t2", name="t2")
                nc.sync.dma_start(t2[:], t_d[s])
                st[s]["xp"] = xp
                st[s]["t2"] = t2

        def stg_gn1s(s, rb):
            src = st[s]["xp"] if rb == "r1" else st[s]["av"]
            with nc.named_scope(f"s{s}_{rb}_gn1s"):
                st[s]["gn1st"] = gn_stats(src)

        def stg_gn1r(s, rb):
            src = st[s]["xp"] if rb == "r1" else st[s]["av"]
            with nc.named_scope(f"s{s}_{rb}_gn1r"):
                a = alloc_padded("act", actp, BF16, zero2b)
                gn_rest(st[s].pop("gn1st"), src, a, rb, 1)
                st[s]["a"] = a

        def stg_conv1(s, rb, rbi):
            t2 = st[s]["t2"]
            with nc.named_scope(f"s{s}_{rb}_conv1"):
                h = alloc_padded("hp", hpad, BF16, zero2b)

                def eat1(ocb, ls, ps):
                    nc.vector.tensor_scalar_add(
                        h[ocb][:, 1 + ls * LS : 1 + (ls + 1) * LS], ps[:],
                        t2[:, ocb, rbi : rbi + 1],
                    )
                conv3(st[s].pop("a"), w1_sb[rb], eat1)
                st[s]["h"] = h

        def stg_gn2s(s, rb):
            with nc.named_scope(f"s{s}_{rb}_gn2s"):
                st[s]["gn2st"] = gn_stats(st[s]["h"])

        def stg_gn2r(s, rb):
            with nc.named_scope(f"s{s}_{rb}_gn2r"):
                a2 = alloc_padded("act", actp, BF16, zero2b)
                gn_rest(st[s].pop("gn2st"), st[s]["h"], a2, rb, 2)
                st[s]["a2"] = a2

        def stg_conv2(s, rb):
            final = rb == "r2"
            src = st[s]["xp"] if rb == "r1" else st[s]["av"]
            with nc.named_scope(f"s{s}_{rb}_conv2"):
                res = None
                if not final:
                    res = [seqp.tile([P, L], BF16, tag=f"res{cb}", name=f"res{cb}") for cb in range(CB)]

                def eat2(ocb, ls, ps):
                    if rb in c2b_sb:
                        nc.vector.tensor_scalar_add(ps[:], ps[:], c2b_sb[rb][:, ocb : ocb + 1])
                    resid = src[ocb][:, 1 + ls * LS : 1 + (ls + 1) * LS]
                    if final:
                        ot = outp.tile([P, LS], F32, tag="out", name="ot")
                        nc.vector.tensor_tensor(out=ot[:], in0=ps[:], in1=resid, op=OP.add)
                        nc.sync.dma_start(
                            out_d[s, ocb * P : (ocb + 1) * P, ls * LS : (ls + 1) * LS], ot[:]
                        )
                    else:
                        nc.vector.tensor_tensor(
                            out=res[ocb][:, ls * LS : (ls + 1) * LS],
                            in0=ps[:], in1=resid, op=OP.add,
                        )
                conv3(st[s].pop("a2"), w2_sb[rb], eat2)
                if not final:
                    st[s]["x1"] = res

        def stg_kqv(s):
            """kqv projections (lin_w rows: [k | q | v]); k/q packed fp8
            [P(c within block), cb, L] for the DoubleRow score matmuls."""
            x1 = st[s].pop("x1")
            kq = {}
            with nc.named_scope(f"s{s}_kqv"):
                for j, nm in ((0, "kt"), (1, "qt")):
                    dst = kqp.tile([P, CB, L], F8E4, tag=nm, name=nm)
                    kq[nm] = dst
                    for ocb in range(CB):
                        off = j * C + ocb * P
                        for ls in range(NL):
                            ps = pacc.tile([P, LS], F32, tag="acc", name="acc")
                            for icb in range(CB):
                                nc.tensor.matmul(
                                    ps[:],
                                    wkqv_sb[:, icb, off : off + P],
                                    x1[icb][:, ls * LS : (ls + 1) * LS],
                                    start=(icb == 0),
                                    stop=(icb == 1),
                                )
                            dsl = dst[:, ocb, ls * LS : (ls + 1) * LS]
                            if linb_sb is not None:
                                nc.vector.tensor_scalar_add(
                                    dsl, ps[:], linb_sb[:, j * CB + ocb : j * CB + ocb + 1]
                                )
                            else:
                                nc.vector.tensor_copy(out=dsl, in_=ps[:])
                # vT[l, c] (l on partitions) with ones col for the denominator
                vt = vtp.tile([P, KB, VPP], F8E4, tag="vt", name="vt")
                nc.vector.memset(vt[:, :, C : C + 1], 1.0)
                for lb in range(KB):
                    ps = pacc.tile([P, LS], F32, tag="acc", name="acc")
                    for icb in range(CB):
                        nc.tensor.matmul(
                            ps[:, :C],
                            x1[icb][:, lb * P : (lb + 1) * P],
                            wkqv_sb[:, icb, 2 * C : 3 * C],
                            start=(icb == 0),
                            stop=(icb == 1),
                        )
                    if linb_sb is not None:
                        for cb in range(CB):
                            nc.vector.tensor_scalar_add(
                                vt[:, lb, cb * P : (cb + 1) * P], ps[:, cb * P : (cb + 1) * P],
                                linb_sb[:, 2 * CB + cb : 2 * CB + cb + 1],
                            )
                    else:
                        nc.scalar.activation(out=vt[:, lb, :C], in_=ps[:, :C], func=AF.Copy)
                st[s]["kt"] = kq["kt"]
                st[s]["qt"] = kq["qt"]
                st[s]["vt"] = vt

        def stg_attn(s):
            """softmax over k: scores[k,q] -> exp (fp8, shifted) ->
            av^T[q, 0:256] + denom[q] at col 256 -> normalize -> transpose."""
            kt, qt, vt = st[s]["kt"], st[s]["qt"], st[s]["vt"]
            av = alloc_padded("hp", hpad, BF16, zero2b)
            exs = {}

            def attn_scores(qs):
                ex = expp.tile([P, KB, LS], F8E4, tag="exp", name="exp")
                for kp in range(KB // 2):
                    ps = scp.tile([P, 2, LS], F32, tag="sc", name="sc")
                    for j in range(2):
                        kb = 2 * kp + j
                        if USE_DR:
                            nc.tensor.matmul(
                                ps[:, j, :],
                                kt[:, :, kb * P : (kb + 1) * P],
                                qt[:, :, qs * LS : (qs + 1) * LS],
                                start=True,
                                stop=True,
                                perf_mode=DRMODE,
                            )
                        else:
                            for cb in range(CB):
                                nc.tensor.matmul(
                                    ps[:, j, :],
                                    kt[:, cb, kb * P : (kb + 1) * P],
                                    qt[:, cb, qs * LS : (qs + 1) * LS],
                                    start=(cb == 0),
                                    stop=(cb == 1),
                                )
                    nc.scalar.activation(
                        out=ex[:, 2 * kp : 2 * kp + 2, :], in_=ps[:, :, :],
                        func=AF.Exp, scale=SCALE, bias=eshift_sb[:],
                    )
                exs[qs] = ex

            def attn_av(qs):
                ex = exs.pop(qs)
                tr = ptrp.tile([P, CB, LS], F32, tag="tr", name="tr")
                for qb in range(QB):
                    avt = pacc.tile([P, LS], F32, tag="acc", name="avt")
                    if AV_DR:
                        for kp in range(KB // 2):
                            nc.tensor.matmul(
                                avt[:, :VP],
                                ex[:, 2 * kp : 2 * kp + 2, qb * P : (qb + 1) * P],
                                vt[:, 2 * kp : 2 * kp + 2, :VP],
                                start=(kp == 0),
                                stop=(kp == KB // 2 - 1),
                                perf_mode=DRMODE,
                            )
                    else:
                        for kb in range(KB):
                            nc.tensor.matmul(
                                avt[:, :VP],
                                ex[:, kb, qb * P : (qb + 1) * P],
                                vt[:, kb, :VP],
                                start=(kb == 0),
                                stop=(kb == KB - 1),
                            )
                    rd = small.tile([P, 1], F32, tag="rd", name="rd")
                    nc.vector.reciprocal(rd[:], avt[:, C : C + 1])
                    avn = avnp.tile([P, C], BF16, tag="avn", name="avn")
                    nc.scalar.activation(out=avn[:], in_=avt[:, :C], func=AF.Copy, scale=rd[:])
                    for cb in range(CB):
                        nc.tensor.matmul(
                            tr[:, cb, qb * P : (qb + 1) * P],
                            avn[:, cb * P : (cb + 1) * P],
                            ident_sb[:],
                            start=True,
                            stop=True,
                        )
                for cb in range(CB):
                    nc.vector.tensor_copy(
                        out=av[cb][:, 1 + qs * LS : 1 + (qs + 1) * LS], in_=tr[:, cb, :]
                    )

            # software-pipeline: scores(qs+1) issues on PE before av^T(qs) so the
            # PE has work while the exp stream for qs drains on ScalarE
            with nc.named_scope(f"s{s}_attn_sc0"):
                attn_scores(0)
            for qs in range(1, NL):
                with nc.named_scope(f"s{s}_attn_sc{qs}"):
                    attn_scores(qs)
                with nc.named_scope(f"s{s}_attn_av{qs - 1}"):
                    attn_av(qs - 1)
            with nc.named_scope(f"s{s}_attn_av{NL - 1}"):
                attn_av(NL - 1)
            st[s]["av"] = av

        # ---- interleaved emission: sample B's convs cover sample A's GNs ----
        assert samples == 2
        A, B = 0, 1

        stg_load(A)
        stg_load(B)
        order = [
            (stg_gn1s, A, "r1"), (stg_gn1r, A, "r1"),
            (stg_gn1s, B, "r1"),
            (stg_conv1, A, "r1", 0),
            (stg_gn1r, B, "r1"),
            (stg_gn2s, A, "r1"),
            (stg_conv1, B, "r1", 0),
            (stg_gn2r, A, "r1"),
            (stg_conv2, A, "r1"),
            (stg_gn2s, B, "r1"), (stg_gn2r, B, "r1"),
            (stg_kqv, A),
            (stg_conv2, B, "r1"),
            (stg_attn, A),
            (stg_gn1s, A, "r2"),
            (stg_kqv, B),
            (stg_gn1r, A, "r2"),
            (stg_attn, B),
            (stg_conv1, A, "r2", 1),
            (stg_gn1s, B, "r2"), (stg_gn1r, B, "r2"),
            (stg_gn2s, A, "r2"),
            (stg_conv1, B, "r2", 1),
            (stg_gn2r, A, "r2"),
            (stg_conv2, A, "r2"),
            (stg_gn2s, B, "r2"), (stg_gn2r, B, "r2"),
            (stg_conv2, B, "r2"),
        ]
        for fn, *args in order:
            fn(*args)
            if DEBUG:
                if fn is stg_gn1r and args[0] == A and args[1] == "r1":
                    nc.sync.dma_start(dbga_d[:], st[A]["a"][0][:, 1 : L + 1])
                if fn is stg_conv1 and args[0] == A and args[1] == "r1":
                    nc.sync.dma_start(dbgh_d[:], st[A]["h"][0][:, 1 : L + 1])
                if fn is stg_kqv and args[0] == A:
                    nc.sync.dma_start(dbgvt_d[:], st[A]["vt"][:])
                if fn is stg_attn and args[0] == A:
                    nc.sync.dma_start(dbgav_d[:], st[A]["av"][0][:, 1 : L + 1])

    nc.finalize()
    return nc


def _pack_conv_w(w):
    """(O, I, 3) -> [P, icb, tap, oc] bf16."""
    w = np.asarray(w, dtype=np.float32)
    o, i, k = w.shape
    r = np.ascontiguousarray(w.transpose(1, 2, 0).reshape(CB, P, 3, o).transpose(1, 0, 2, 3))
    return r.astype(_mld.bfloat16)


def _pack_gn(v):
    """(256,) -> [P, CB]"""
    return np.ascontiguousarray(np.asarray(v, dtype=np.float32).reshape(CB, P).T)


def make_in_maps(inp, use_bias):
    """Host-side packing; returns the per-core input maps."""
    gind = np.zeros((P, GPB), np.float32)
    bind = np.zeros((CB, P, P), np.float32)
    for cc in range(P):
        gind[cc, cc // 8] = 0.125
        for cb in range(CB):
            bind[cb, cb * 32 + cc // 8, cc] = 1.0
    shared = {
        "wkqvt": np.ascontiguousarray(
            inp["lin_w"][:, :, 0].T.reshape(CB, P, 3 * C).transpose(1, 0, 2)
        ).astype(_mld.bfloat16),
        "gind": gind.astype(_mld.bfloat16),
        "bind": bind.astype(_mld.bfloat16),
        "ident": np.eye(P, dtype=_mld.bfloat16),
    }
    for rb in ("r1", "r2"):
        shared[f"{rb}_w1t"] = _pack_conv_w(inp[f"{rb}_c1_w"])
        shared[f"{rb}_w2t"] = _pack_conv_w(inp[f"{rb}_c2_w"])
        for ln in (1, 2):
            shared[f"{rb}_gn{ln}_ws"] = _pack_gn(inp[f"{rb}_gn{ln}_w"])
            shared[f"{rb}_gn{ln}_bs"] = _pack_gn(inp[f"{rb}_gn{ln}_b"])
    if "c2b_r1" in use_bias:
        shared["r1_c2bs"] = _pack_gn(inp["r1_c2_b"])
    if "c2b_r2" in use_bias:
        shared["r2_c2bs"] = _pack_gn(inp["r2_c2_b"])
    if "linb" in use_bias:
        shared["lin_bs"] = np.ascontiguousarray(inp["lin_b"].reshape(3 * CB, P).T)

    # per-sample conv1 bias vector: t[s] + c1_b per res block -> [P, CB, 2]
    tfull = inp["t"][:, :, 0]  # (B, C)
    nb = inp["x"].shape[0]
    tv = np.empty((nb, P, CB, 2), np.float32)
    for rbi, rb in enumerate(("r1", "r2")):
        v = tfull + inp[f"{rb}_c1_b"][None, :]
        tv[:, :, :, rbi] = v.reshape(nb, CB, P).transpose(0, 2, 1)

    in_maps = []
    for c in range(NCORES):
        sl = slice(S * c, S * (c + 1))
        m = dict(shared)
        m["x"] = inp["x"][sl]
        m["tv"] = np.ascontiguousarray(tv[sl])
        in_maps.append(m)
    return in_maps


_CACHE = {}


def kernel(**inputs):
    inp = {k: np.ascontiguousarray(np.asarray(v, dtype=np.float32)) for k, v in inputs.items()}

    use_bias = []
    if np.any(inp["r1_c2_b"]):
        use_bias.append("c2b_r1")
    if np.any(inp["r2_c2_b"]):
        use_bias.append("c2b_r2")
    if np.any(inp["lin_b"]):
        use_bias.append("linb")
    use_bias = tuple(use_bias)

    if ("nc", use_bias) not in _CACHE:
        _CACHE[("nc", use_bias)] = build_program(S, use_bias)
    nc = _CACHE[("nc", use_bias)]

    in_maps = make_in_maps(inp, use_bias)
    res = _bu.run_bass_kernel_spmd(nc, in_maps, core_ids=list(range(NCORES)))
    out = np.concatenate([res.results[c]["out"] for c in range(NCORES)], axis=0)
    return out.astype(np.float32)
